# revision 55
# baseline (speedup 1.0000x reference)
"""CCA-SSG (2-layer GCN backbone x2 graphs + z-score) on 8 Trainium2 NeuronCores.

Strategy (graph/data parallel, per sharding hint):
  - Nodes row-sharded across 8 cores (12500/core). Edges routed to the core
    owning their destination. Weights replicated.
  - Algebraic restructure: with g = (x @ W) * dinv,  GCNConv output is
        out[d] = dinv[d] * (sum_{e: dst=d} g[src[e]] + g[d]) + b
    so the edge aggregation is an UNWEIGHTED segment-sum of gathered rows.
  - Per layer: compute local g shard -> AllGather full g table (HBM) ->
    dma_gather 256B rows by src -> one-hot matmul segment-sum into PSUM
    (S[e, j] = (dstloc[e] == j), agg = S^T @ G accumulated per 128-node block).
  - Gather table is split in 4 banks of <=25000 rows (dma_gather uses int16
    indices, read from SBUF partitions 16-31 on HW / 0-15 in CoreSim).
  - mean/std over nodes: per-core partial sum/sumsq via ones-matmul,
    AllReduce, broadcast back via K=1 matmul.

Host side does only sharding/routing work: edge bucketing by (bank, block),
padding, int16 index packing, x transpose-blocking, degree bincount.
"""
import math
import os
import sys

sys.path.insert(0, "/opt/trn_rl_repo")

import numpy as np

import concourse.bacc as bacc
import concourse.bass as bass
import concourse.mybir as mybir
import concourse.tile as tile
from concourse.bass_utils import run_bass_kernel_spmd

P = 128
CORES = 8
IN_DIM = 256
HID = 64  # = OUT_DIM; both layers have width 64
BANKS = 4
GCHUNK = 8    # chunks per dma_gather (num_idxs limit: >1024 crashes the Q7 ucode; re-verified)
SGROUP = 16   # chunks per is_equal S-build op

F32 = mybir.dt.float32
I16 = mybir.dt.int16

LAST_EXEC_NS = None


# ----------------------------------------------------------------------------
# host-side sharding / routing
# ----------------------------------------------------------------------------

def _block_sizes(npc_real):
    """Uniform-ish block real-row sizes per piece. Returns (sizes[nblk], BP)."""
    pr = npc_real // BANKS
    bp = -(-pr // 125)
    br = -(-pr // bp)
    sz_piece = [br] * (bp - 1) + [pr - br * (bp - 1)]
    return np.array(sz_piece * BANKS, np.int64), bp


def _balance(src, dst, n_nodes, npc_real):
    """Per-core, per-piece node permutation balancing per-(bank, block)
    in-degree sums. A node's piece (= its table bank) is fixed by its
    original within-core rank, so bank membership is independent of the
    permutation. Positions are PADDED: block b occupies slots
    [b*128, b*128+size_b); pad slots stay empty.

    Returns pos[n_nodes]: padded global position of each node."""
    cores = n_nodes // npc_real
    pr = npc_real // BANKS  # real nodes per piece
    sizes_all, bp = _block_sizes(npc_real)
    npc_pad = len(sizes_all) * P

    q = (src % npc_real) // pr  # bank == original piece, invariant by construction
    v = np.zeros((n_nodes, BANKS), np.int64)
    np.add.at(v, (dst, q), 1)

    pos = np.empty(n_nodes, np.int64)
    for c in range(cores):
        for piece in range(BANKS):
            ids = np.arange(c * npc_real + piece * pr,
                            c * npc_real + (piece + 1) * pr)
            vc = v[ids]
            sizes = sizes_all[piece * bp : (piece + 1) * bp]
            wheel_pos = np.concatenate(
                [np.flatnonzero(sizes > j) * P + j
                 for j in range(int(sizes.max()))])
            _t, inv = np.unique(vc, axis=0, return_inverse=True)
            order = np.argsort(inv, kind="stable")
            p = np.empty(pr, np.int64)
            p[order] = wheel_pos
            blk = _swap_search(vc.astype(np.float64), p // P, sizes, pr, bp)
            bo = np.argsort(blk, kind="stable")
            rank = np.arange(pr) - np.repeat(
                np.concatenate([[0], np.cumsum(np.bincount(blk, minlength=bp))[:-1]]),
                np.bincount(blk, minlength=bp))
            p2 = np.empty(pr, np.int64)
            p2[bo] = blk[bo] * P + rank
            pos[ids] = c * npc_pad + piece * bp * P + p2
    return pos


def _swap_search(vc, blk, sizes, npc, nblk, rounds=25, K=30000, seed=0):
    """Pairwise-swap local search: minimize sum of squared per-(block, bank)
    deviations from size-proportional targets. Swaps preserve block sizes."""
    rng = np.random.default_rng(seed)
    tgt = sizes[:, None] * (vc.sum(axis=0)[None, :] / npc)

    def dev_of(b):
        s = np.zeros((nblk, vc.shape[1]))
        np.add.at(s, b, vc)
        return s - tgt

    dev = dev_of(blk)
    for _ in range(rounds):
        i = rng.integers(0, npc, K)
        j = rng.integers(0, npc, K)
        bi, bj = blk[i], blk[j]
        m = bi != bj
        i, j, bi, bj = i[m], j[m], bi[m], bj[m]
        delta = vc[j] - vc[i]
        dobj = (((dev[bi] + delta) ** 2).sum(1) + ((dev[bj] - delta) ** 2).sum(1)
                - (dev[bi] ** 2).sum(1) - (dev[bj] ** 2).sum(1))
        good = dobj < -1e-9
        i, j, bi, bj, dobj = i[good], j[good], bi[good], bj[good], dobj[good]
        o = np.argsort(dobj)
        i, j, bi, bj = i[o], j[o], bi[o], bj[o]
        used = np.zeros(nblk, bool)
        un = np.zeros(npc, bool)
        acc = []
        for k in range(len(i)):
            if used[bi[k]] or used[bj[k]] or un[i[k]] or un[j[k]]:
                continue
            used[bi[k]] = used[bj[k]] = True
            un[i[k]] = un[j[k]] = True
            acc.append(k)
        if not acc:
            break
        acc = np.asarray(acc, np.int64)
        ii, jj = i[acc], j[acc]
        blk[ii], blk[jj] = blk[jj], blk[ii]
        dev = dev_of(blk)
    return blk


def _route_graph(src, dst, n_nodes, npc, nblk, bank_rows):
    """Route edges by destination core; bucket by (bank(src), block(dst)).

    Packed layout: within each bank, bucket segments are laid out back to back
    (segment length = max bucket count over cores), so chunks may straddle
    bucket boundaries. A straddling chunk gets one "sentry" (S-matrix column
    group + matmul) per bucket it overlaps; out-of-bucket rows are poisoned
    with dstloc=255 so the one-hot never matches them.

    Returns dict(totch, specs, sentries) + per-core (idx_stream, dstl [nsent,P])."""
    cores = n_nodes // npc
    per_core = []
    counts = np.zeros((cores, BANKS, nblk), np.int64)
    for c in range(cores):
        lo, hi = c * npc, (c + 1) * npc
        m = (dst >= lo) & (dst < hi)
        s = src[m]
        dl = (dst[m] - lo).astype(np.int64)
        blk = dl >> 7
        q = s // bank_rows
        order = np.lexsort((s, blk, q))
        s, dl, blk, q = s[order], dl[order], blk[order], q[order]
        np.add.at(counts[c], (q, blk), 1)
        per_core.append((s, dl))

    seg = counts.max(axis=0)  # [BANKS, nblk] shared bucket segment lengths
    bank_nch = [int(-(-int(seg[q].sum()) // P)) for q in range(BANKS)]
    bank_c0 = np.concatenate([[0], np.cumsum(bank_nch)]).astype(np.int64)
    totch = int(bank_c0[-1])
    bucket_off = np.zeros((BANKS, nblk), np.int64)
    for q in range(BANKS):
        bucket_off[q] = bank_c0[q] * P + np.concatenate(
            [[0], np.cumsum(seg[q])[:-1]])

    # sentries: (chunk, block, is_first_of_run, is_last_of_run)
    sentries = []
    for q in range(BANKS):
        for b in range(nblk):
            o, n = int(bucket_off[q, b]), int(seg[q, b])
            if n == 0:
                continue
            c_first, c_last = o // P, (o + n - 1) // P
            for ci in range(c_first, c_last + 1):
                sentries.append((ci, b, ci == c_first, ci == c_last))
    nsent = len(sentries)

    # gather specs: per bank, chunks in groups of <= GCHUNK
    specs = []
    for q in range(BANKS):
        done = 0
        while done < bank_nch[q]:
            k = min(GCHUNK, bank_nch[q] - done)
            specs.append((q, int(bank_c0[q]) + done, k))
            done += k

    out = []
    for c in range(cores):
        s, dl = per_core[c]
        cnt = counts[c]
        idx_stream = np.zeros(totch * P, np.int16)
        dst_pos = np.full(totch * P, 255.0, np.float32)
        pos_in = 0
        for q in range(BANKS):
            for b in range(nblk):
                n = int(cnt[q, b])
                if n == 0:
                    continue
                o = int(bucket_off[q, b])
                idx_stream[o : o + n] = (
                    s[pos_in : pos_in + n] - q * bank_rows).astype(np.int16)
                dst_pos[o : o + n] = (
                    dl[pos_in : pos_in + n] - b * P).astype(np.float32)
                pos_in += n
        assert pos_in == len(s)
        # per-sentry dstloc columns, masked to the sentry's bucket span
        dstl = np.full((nsent, P), 255.0, np.float32)
        for j, (ci, b, _st, _sp) in enumerate(sentries):
            q = int(np.searchsorted(bank_c0, ci, side="right")) - 1
            o, n = int(bucket_off[q, b]), int(seg[q, b])
            lo_p = max(o, ci * P)
            hi_p = min(o + n, (ci + 1) * P)
            dstl[j, lo_p - ci * P : hi_p - ci * P] = dst_pos[lo_p:hi_p]
        out.append((idx_stream, dstl))
    return {"totch": totch, "specs": specs, "sentries": sentries}, out


def _pack_idx16(idx_stream, specs):
    """[128, totch*8] int16: per gather instruction local index j lives at
    row 16 + j%16 (HW) and j%16 (CoreSim), column c0*8 + j//16."""
    totch = len(idx_stream) // P
    arr = np.zeros((P, totch * 8), np.int16)
    for (_q, c0, nch) in specs:
        seg = idx_stream[c0 * P : (c0 + nch) * P]
        w = seg.reshape(-1, 16).T  # [16, nch*8]
        arr[0:16, c0 * 8 : (c0 + nch) * 8] = w
        arr[16:32, c0 * 8 : (c0 + nch) * 8] = w
    return arr


# ----------------------------------------------------------------------------
# device kernel builder
# ----------------------------------------------------------------------------

def _build_nc(n_real, npc, nblk, bank_rows, sizes_all, bp, tables, split=True):
    """tables: per graph dict(totch, specs, sentries). npc is padded; each
    block holds sizes_all[b] real rows; bp blocks form one piece (= bank)."""
    npc_pad = nblk * P
    piece_rows = bp * P

    nc = bacc.Bacc(None, target_bir_lowering=False, debug=False)

    # ---- parameters (per core) ----
    XB = 7  # x-tile DMA batch (blocks per load); nblk padded to a multiple
    ngrp = -(-nblk // XB)
    xtb = [nc.declare_dram_parameter(f"xtb{g}", [ngrp, P, 2 * XB * P], F32, isOutput=False)
           for g in range(2)]
    deg_in = [nc.declare_dram_parameter(f"deg{g}", [P, nblk], F32, isOutput=False)
              for g in range(2)]
    dstl_in = [nc.declare_dram_parameter(f"dstloc{g}", [P, len(tables[g]["sentries"])], F32, isOutput=False)
               for g in range(2)]
    idx_in = [nc.declare_dram_parameter(f"idx{g}", [P, tables[g]["totch"] * 8], I16, isOutput=False)
              for g in range(2)]
    w1p_in = nc.declare_dram_parameter("w1p", [P, 2 * HID], F32, isOutput=False)
    w2_in = nc.declare_dram_parameter("w2", [HID, HID], F32, isOutput=False)
    b1_in = nc.declare_dram_parameter("b1t", [P, HID], F32, isOutput=False)
    b2_in = nc.declare_dram_parameter("b2t", [P, HID], F32, isOutput=False)
    iota_in = nc.declare_dram_parameter("iota", [P, P], F32, isOutput=False)
    ident_in = nc.declare_dram_parameter("ident", [P, P], F32, isOutput=False)
    ones_in = nc.declare_dram_parameter("ones", [P, P], F32, isOutput=False)
    zout = nc.declare_dram_parameter("zout", [2, P, nblk * HID], F32, isOutput=True)

    # ---- internal DRAM ----
    g_shard = [[[nc.dram_tensor(f"gshard{g}_{l}_{k}", [piece_rows, HID], F32)
                 for k in range(BANKS)] for l in range(2)] for g in range(2)]
    g_full = [[[nc.dram_tensor(f"gfull{g}_{l}_{k}", [bank_rows, HID], F32,
                               addr_space="Shared")
                for k in range(BANKS)] for l in range(2)] for g in range(2)]
    stats_in = nc.dram_tensor("stats_in", [1, 4 * HID], F32)
    stats_out = nc.dram_tensor("stats_out", [1, 4 * HID], F32, addr_space="Shared")

    rg = [list(range(CORES))]

    with tile.TileContext(nc) as tc:
        with (
            tc.tile_pool(name="const", bufs=1) as cpool,
            tc.tile_pool(name="acc", bufs=1) as apool,
            tc.tile_pool(name="work", bufs=3) as wpool,
            tc.tile_pool(name="blk", bufs=4) as bpool,
            tc.tile_pool(name="psA", bufs=2, space="PSUM") as psA,
            tc.tile_pool(name="psTr", bufs=1, space="PSUM") as psTr,
            tc.tile_pool(name="psAgg", bufs=3, space="PSUM") as psAgg,
            tc.tile_pool(name="psSm", bufs=1, space="PSUM") as psSm,
        ):
            # ---- constants ----
            w1p = cpool.tile([P, 2 * HID], F32)
            nc.sync.dma_start(w1p[:], w1p_in[:])
            w2sb = cpool.tile([HID, HID], F32)
            nc.sync.dma_start(w2sb[:], w2_in[:])
            b1sb = cpool.tile([P, HID], F32)
            nc.sync.dma_start(b1sb[:], b1_in[:])
            b2sb = cpool.tile([P, HID], F32)
            nc.sync.dma_start(b2sb[:], b2_in[:])
            iota = cpool.tile([P, P], F32)
            nc.sync.dma_start(iota[:], iota_in[:])
            ident = cpool.tile([P, P], F32)
            nc.sync.dma_start(ident[:], ident_in[:])
            ones = cpool.tile([P, P], F32)
            nc.sync.dma_start(ones[:], ones_in[:])
            ones_col = ones[:, 0:1]         # [128, 1] of ones
            ones_row = ones[0:1, :]         # [1, 128] of ones

            dinv = []
            for g in range(2):
                dt = cpool.tile([P, nblk], F32, tag=f"deg{g}")
                nc.sync.dma_start(dt[:], deg_in[g][:])
                sq = cpool.tile([P, nblk], F32, tag=f"dsq{g}")
                nc.scalar.activation(sq[:], dt[:], mybir.ActivationFunctionType.Sqrt)
                dv = cpool.tile([P, nblk], F32, tag=f"dinv{g}")
                nc.vector.reciprocal(dv[:], sq[:])
                dinv.append(dv)

            accB = [apool.tile([P, nblk * HID], F32, tag=f"accB{g}", name=f"accB{g}") for g in range(2)]
            accC = [apool.tile([P, nblk * HID], F32, tag=f"accC{g}", name=f"accC{g}") for g in range(2)]

            # ---- phase A: g0 = (x @ W1) * dinv, piecewise allgather ----
            for g in range(2):
                for b in range(nblk):
                    if b % XB == 0:
                        xt4 = wpool.tile([P, 2 * XB * P], F32, tag="xt", bufs=2)
                        nc.sync.dma_start(xt4[:], xtb[g][b // XB])
                    ph = psA.tile([P, HID], F32, tag="hps")
                    for k in range(2):
                        ko = ((b % XB) * 2 + k) * P
                        xt = xt4[:, ko : ko + P]
                        nc.tensor.matmul(
                            out=ph[:], lhsT=xt, rhs=w1p[:, k * HID : (k + 1) * HID],
                            start=(k == 0), stop=(k == 1))
                    gblk = accB[g][:, b * HID : (b + 1) * HID]
                    nc.scalar.activation(gblk, ph[:],
                                         mybir.ActivationFunctionType.Copy,
                                         scale=dinv[g][:, b : b + 1])
                    nc.sync.dma_start(
                        g_shard[g][0][b // bp][(b % bp) * P : (b % bp + 1) * P, :],
                        gblk)
                    if b % bp == bp - 1:
                        k_ = b // bp
                        nc.gpsimd.collective_compute(
                            "AllGather", mybir.AluOpType.bypass, replica_groups=rg,
                            ins=[g_shard[g][0][k_][:]], outs=[g_full[g][0][k_][:]])

            # ---- aggregation emitter ----
            dstl_tiles = {}
            for g in range(2):
                dt_ = cpool.tile([P, len(tables[g]["sentries"])], F32, tag=f"dstl{g}")
                nc.sync.dma_start(dt_[:], dstl_in[g][:])
                dstl_tiles[g] = dt_

            def aggregate(g, layer, acc):
                """acc[:, b*64:(b+1)*64] += segment_sum of gathered g rows."""
                if int(os.environ.get("KERNEL_NO_AGG", "0")):
                    return
                t = tables[g]
                specs, sentries = t["specs"], t["sentries"]
                dstl = dstl_tiles[g]
                banks = g_full[g][layer]
                gt = {}
                spec_i = 0
                stile = None
                sbase = 0
                ps = None
                nsent = len(sentries)
                for j, (ci, b, st, sp) in enumerate(sentries):
                    # emit every gather whose chunk range we have now reached
                    while spec_i < len(specs) and specs[spec_i][1] <= ci:
                        q, c0, nch = specs[spec_i]
                        it = wpool.tile([P, GCHUNK * 8], I16, tag="idx", bufs=6)
                        nc.sync.dma_start(it[:, : nch * 8], idx_in[g][:, c0 * 8 : (c0 + nch) * 8])
                        gtile = wpool.tile([P, GCHUNK * HID], F32, tag="gt", bufs=6)
                        nc.gpsimd.dma_gather(
                            gtile[:, : nch * HID].rearrange("p (c d) -> p c d", c=nch),
                            banks[q][:],
                            it[:, : nch * 8], nch * P, nch * P, HID)
                        gt = {"tile": gtile, "c0": c0}
                        spec_i += 1
                    if j % SGROUP == 0:
                        ns = min(SGROUP, nsent - j)
                        stile = wpool.tile([P, SGROUP * P], F32, tag="stile")
                        s3 = stile[:, : ns * P].rearrange("p (c j) -> p c j", c=ns)
                        nc.vector.tensor_tensor(
                            out=s3,
                            in0=dstl[:, j : j + ns][:, :, None].to_broadcast([P, ns, P]),
                            in1=iota[:, None, :].to_broadcast([P, ns, P]),
                            op=mybir.AluOpType.is_equal)
                        sbase = j
                    if st:
                        ps = psAgg.tile([P, HID], F32, tag="aggps")
                    co = ci - gt["c0"]
                    nc.tensor.matmul(
                        out=ps[:],
                        lhsT=stile[:, (j - sbase) * P : (j - sbase + 1) * P],
                        rhs=gt["tile"][:, co * HID : (co + 1) * HID],
                        start=st, stop=sp, skip_group_check=True)
                    if sp:
                        sl = acc[:, b * HID : (b + 1) * HID]
                        nc.vector.tensor_tensor(out=sl, in0=sl, in1=ps[:],
                                                op=mybir.AluOpType.add)

            # ---- phase B: layer-1 aggregation, relu, @W2, allgather ----
            for g in range(2):
                aggregate(g, 0, accB[g])
                for b in range(nblk):
                    sl = accB[g][:, b * HID : (b + 1) * HID]
                    t1 = bpool.tile([P, HID], F32, tag="t1")
                    nc.scalar.activation(t1[:], sl, mybir.ActivationFunctionType.Copy,
                                         scale=dinv[g][:, b : b + 1])
                    t2 = bpool.tile([P, HID], F32, tag="t2")
                    nc.vector.tensor_tensor(out=t2[:], in0=t1[:], in1=b1sb[:],
                                            op=mybir.AluOpType.add)
                    r = bpool.tile([P, HID], F32, tag="t3")
                    nc.scalar.activation(r[:], t2[:], mybir.ActivationFunctionType.Relu)
                    trp = psTr.tile([HID, P], F32, tag="trps")
                    nc.tensor.transpose(out=trp[:], in_=r[:], identity=ident[:])
                    trs = bpool.tile([HID, P], F32, tag="trs")
                    nc.vector.tensor_copy(trs[:], trp[:])
                    p2 = psA.tile([P, HID], F32, tag="hps")
                    nc.tensor.matmul(out=p2[:], lhsT=trs[:], rhs=w2sb[:],
                                     start=True, stop=True)
                    g2b = accC[g][:, b * HID : (b + 1) * HID]
                    nc.scalar.activation(g2b, p2[:], mybir.ActivationFunctionType.Copy,
                                         scale=dinv[g][:, b : b + 1])
                    nc.sync.dma_start(
                        g_shard[g][1][b // bp][(b % bp) * P : (b % bp + 1) * P, :],
                        g2b)
                    if b % bp == bp - 1:
                        k_ = b // bp
                        nc.gpsimd.collective_compute(
                            "AllGather", mybir.AluOpType.bypass, replica_groups=rg,
                            ins=[g_shard[g][1][k_][:]], outs=[g_full[g][1][k_][:]])

            # ---- phase C: layer-2 aggregation, out2, stats ----
            stats_sb = cpool.tile([1, 4 * HID], F32, tag="stats_sb")
            for g in range(2):
                aggregate(g, 1, accC[g])
                pst = psSm.tile([1, 2 * HID], F32, tag="pstats", name="pst")
                for b in range(nblk):
                    sl = accC[g][:, b * HID : (b + 1) * HID]
                    t1 = bpool.tile([P, HID], F32, tag="t1")
                    nc.scalar.activation(t1[:], sl, mybir.ActivationFunctionType.Copy,
                                         scale=dinv[g][:, b : b + 1])
                    o2sq = bpool.tile([P, 2 * HID], F32, tag="t2")
                    o2 = o2sq[:, :HID]
                    sq = o2sq[:, HID:]
                    nc.vector.tensor_tensor(out=o2, in0=t1[:], in1=b2sb[:],
                                            op=mybir.AluOpType.add)
                    nc.vector.tensor_tensor(out=sq, in0=o2, in1=o2,
                                            op=mybir.AluOpType.mult)
                    # overwrite accC block in place with the final conv2 output
                    nc.scalar.activation(sl, o2, mybir.ActivationFunctionType.Copy)
                    rr = int(sizes_all[b])
                    nc.tensor.matmul(out=pst[:], lhsT=ones_col[:rr], rhs=o2sq[:rr, :],
                                     start=(b == 0), stop=(b == nblk - 1),
                                     skip_group_check=True)
                nc.vector.tensor_copy(stats_sb[:, 2 * HID * g : 2 * HID * (g + 1)], pst[:])
            nc.sync.dma_start(stats_in[:], stats_sb[:])
            nc.gpsimd.collective_compute(
                "AllReduce", mybir.AluOpType.add, replica_groups=rg,
                ins=[stats_in[:]], outs=[stats_out[:]])
            stats_rx = cpool.tile([1, 4 * HID], F32, tag="stats_rx")
            nc.sync.dma_start(stats_rx[:], stats_out[:])

            # ---- z-score ----
            n_f = float(n_real)
            for g in range(2):
                srow = stats_rx[:, 2 * HID * g : 2 * HID * g + HID]
                qrow = stats_rx[:, 2 * HID * g + HID : 2 * HID * (g + 1)]
                mean = cpool.tile([1, HID], F32, tag=f"mean{g}")
                nc.scalar.activation(mean[:], srow, mybir.ActivationFunctionType.Copy,
                                     scale=1.0 / n_f)
                s2 = cpool.tile([1, HID], F32, tag=f"s2_{g}")
                nc.vector.tensor_tensor(out=s2[:], in0=srow, in1=srow,
                                        op=mybir.AluOpType.mult)
                s2n = cpool.tile([1, HID], F32, tag=f"s2n{g}")
                nc.scalar.activation(s2n[:], s2[:], mybir.ActivationFunctionType.Copy,
                                     scale=1.0 / n_f)
                v = cpool.tile([1, HID], F32, tag=f"v{g}")
                nc.vector.tensor_tensor(out=v[:], in0=qrow, in1=s2n[:],
                                        op=mybir.AluOpType.subtract)
                stdv = cpool.tile([1, HID], F32, tag=f"std{g}")
                nc.scalar.activation(stdv[:], v[:], mybir.ActivationFunctionType.Sqrt,
                                     scale=1.0 / (n_f - 1.0))
                rstd = cpool.tile([1, HID], F32, tag=f"rstd{g}")
                nc.vector.reciprocal(rstd[:], stdv[:])
                pb = psSm.tile([P, 2 * HID], F32, tag="bcast")
                pm = pb[:, :HID]
                pr = pb[:, HID:]
                nc.tensor.matmul(out=pm, lhsT=ones_row, rhs=mean[:],
                                 start=True, stop=True, skip_group_check=True)
                nc.tensor.matmul(out=pr, lhsT=ones_row, rhs=rstd[:],
                                 start=True, stop=True, skip_group_check=True)
                ZB = 14  # blocks per z-score slab (98 = 7*14)
                for s in range(0, nblk, ZB):
                    nb2 = min(ZB, nblk - s)
                    slab = accC[g][:, s * HID : (s + nb2) * HID]
                    s3 = slab.rearrange("p (c f) -> p c f", c=nb2)
                    z1 = bpool.tile([P, ZB * HID], F32, tag="z1", bufs=2)
                    z13 = z1[:, : nb2 * HID].rearrange("p (c f) -> p c f", c=nb2)
                    nc.vector.tensor_tensor(
                        out=z13, in0=s3,
                        in1=pm[:, None, :].to_broadcast([P, nb2, HID]),
                        op=mybir.AluOpType.subtract)
                    z2 = bpool.tile([P, ZB * HID], F32, tag="z2", bufs=2)
                    z23 = z2[:, : nb2 * HID].rearrange("p (c f) -> p c f", c=nb2)
                    nc.vector.tensor_tensor(
                        out=z23, in0=z13,
                        in1=pr[:, None, :].to_broadcast([P, nb2, HID]),
                        op=mybir.AluOpType.mult)
                    nc.sync.dma_start(
                        zout[g, :, s * HID : (s + nb2) * HID],
                        z2[:, : nb2 * HID])

    nc.compile()
    if split:
        _split_waits(nc, max_waits=1)
    return nc


# ----------------------------------------------------------------------------
# wait-splitting post-pass (walrus rejects >1 sync wait per instruction here)
# ----------------------------------------------------------------------------

def _split_waits(nc, max_waits=1):
    inserted = 0
    for blk in nc.main_func.blocks:
        bb = blk if hasattr(blk, "instructions") else blk.bb
        new_list = []
        for ins in bb.instructions:
            si = ins.sync_info
            waits = list(si.on_wait) if (si and si.on_wait) else []
            if len(waits) > max_waits:
                keep = waits[-max_waits:]
                extra = waits[:-max_waits]
                for i in range(0, len(extra), max_waits):
                    chunk = extra[i : i + max_waits]
                    nop = mybir.InstNoOp(
                        name=nc.get_next_instruction_name(),
                        engine=ins.engine, ins=[], outs=[], text_hint="wait_split")
                    nop.sync_info = mybir.SyncInfo(on_wait=chunk, on_update=[])
                    new_list.append(nop)
                    inserted += 1
                si.on_wait = keep
            new_list.append(ins)
        bb.instructions[:] = new_list
    return inserted


# ----------------------------------------------------------------------------
# host wrapper
# ----------------------------------------------------------------------------

def _prepare(x1, edge_index1, x2, edge_index2, W1, b1, W2, b2, n_nodes):
    npc_real = n_nodes // CORES
    sizes_all, bp = _block_sizes(npc_real)
    nblk = BANKS * bp
    npc = nblk * P            # padded per-core aggregation space
    npc_pad = npc
    n_pad = CORES * npc
    piece_rows = bp * P       # padded rows per (core, piece)
    bank_rows = CORES * piece_rows  # rows per bank table tensor
    assert bank_rows <= 32767

    graphs = [(np.asarray(x1), np.asarray(edge_index1)),
              (np.asarray(x2), np.asarray(edge_index2))]
    tables = []
    per_core_arrays = [dict() for _ in range(CORES)]
    for g, (x, ei) in enumerate(graphs):
        src0 = np.asarray(ei[0], dtype=np.int64)
        dst0 = np.asarray(ei[1], dtype=np.int64)
        pos = _balance(src0, dst0, n_nodes, npc_real)  # padded positions
        # table coordinates: bank-major [piece][core][within-piece]
        core_of = pos // npc
        w = pos - core_of * npc
        piece = w // piece_rows
        tablepos = piece * bank_rows + core_of * piece_rows + (w - piece * piece_rows)
        src = tablepos[src0]
        dst = pos[dst0]
        deg = np.bincount(dst, minlength=n_pad).astype(np.float32) + 1.0
        table, routed = _route_graph(src, dst, n_pad, npc, nblk, bank_rows)
        table["pos"] = pos
        tables.append(table)
        x = np.asarray(x, dtype=np.float32)
        for c in range(CORES):
            idx_stream, dstl = routed[c]
            d = per_core_arrays[c]
            XB = 7
            ngrp = -(-nblk // XB)
            xp = np.zeros((ngrp * XB * P, IN_DIM), np.float32)
            in_core = np.flatnonzero(core_of == c)
            xp[w[in_core]] = x[in_core]
            # [grp, q, b_in_grp, k, p] so each group is one 2D SBUF-layout DMA
            d[f"xtb{g}"] = np.ascontiguousarray(
                xp.reshape(ngrp, XB, P, 2, P).transpose(0, 4, 1, 3, 2)
                .reshape(ngrp, P, 2 * XB * P))
            degp = deg[c * npc : (c + 1) * npc]
            d[f"deg{g}"] = np.ascontiguousarray(degp.reshape(nblk, P).T)
            d[f"dstloc{g}"] = np.ascontiguousarray(dstl.T)
            d[f"idx{g}"] = _pack_idx16(idx_stream, table["specs"])

    W1 = np.asarray(W1, np.float32)
    w1p = np.zeros((P, 2 * HID), np.float32)
    w1p[:, :HID] = W1[:P]
    w1p[:, HID:] = W1[P:]
    shared = {
        "w1p": w1p,
        "w2": np.asarray(W2, np.float32),
        "b1t": np.broadcast_to(np.asarray(b1, np.float32), (P, HID)).copy(),
        "b2t": np.broadcast_to(np.asarray(b2, np.float32), (P, HID)).copy(),
        "iota": np.tile(np.arange(P, dtype=np.float32), (P, 1)),
        "ident": np.eye(P, dtype=np.float32),
        "ones": np.ones((P, P), np.float32),
    }
    for d in per_core_arrays:
        d.update(shared)
    return tables, per_core_arrays, npc, nblk, bank_rows, sizes_all, bp



def _install_profile_shim():
    """ctypes NTFF hook for run_bass_kernel_spmd(trace=True) under axon."""
    import contextlib
    import ctypes
    import types
    if "antenv.axon_hooks" in sys.modules:
        return
    try:
        lib = ctypes.CDLL("/opt/axon/libaxon_pjrt.so")
        lib.axon_start_nrt_profile.argtypes = [ctypes.POINTER(ctypes.c_int64), ctypes.c_size_t]
        lib.axon_start_nrt_profile.restype = ctypes.c_int64
        lib.axon_stop_nrt_profile.argtypes = [ctypes.c_char_p]
        lib.axon_stop_nrt_profile.restype = ctypes.c_int64
    except (OSError, AttributeError):
        return

    @contextlib.contextmanager
    def _hook(output_dir, device_ids):
        import jax
        jax.devices()
        if device_ids:
            ids = (ctypes.c_int64 * len(device_ids))(*device_ids)
            rc = lib.axon_start_nrt_profile(ids, len(device_ids))
        else:
            rc = lib.axon_start_nrt_profile(None, 0)
        if rc != 0:
            raise RuntimeError(f"axon_start_nrt_profile rc={rc}")
        try:
            yield
        finally:
            n = lib.axon_stop_nrt_profile(str(output_dir).encode())
            print(f"ntff profile: {n} file(s) -> {output_dir}", file=sys.stderr)

    mod = types.ModuleType("antenv.axon_hooks")
    mod.get_axon_ntff_profile_hook = lambda: _hook
    mod.set_axon_ntff_profile_hook = lambda h: None
    sys.modules["antenv.axon_hooks"] = mod

    from concourse import bass_utils
    bass_utils.upload_artifacts = lambda tmpdir: f"local:{tmpdir}"

_NC_CACHE = {}


def _run(x1, edge_index1, x2, edge_index2, W1, b1, W2, b2, n_nodes, trace=False):
    global LAST_EXEC_NS
    tables, in_maps, npc, nblk, bank_rows, sizes_all, bp = _prepare(
        x1, edge_index1, x2, edge_index2, W1, b1, W2, b2, n_nodes)

    sim_mode = bool(int(os.environ.get("KERNEL_SIM", "0")))
    key = (n_nodes, sim_mode,
           tuple(tables[0]["sentries"]), tuple(tables[0]["specs"]),
           tuple(tables[1]["sentries"]), tuple(tables[1]["specs"]))
    if key not in _NC_CACHE:
        _NC_CACHE[key] = _build_nc(n_nodes, npc, nblk, bank_rows, sizes_all,
                                   bp, tables, split=not sim_mode)
    nc = _NC_CACHE[key]

    def _unscramble(zraw):
        # [2, P, nblk*HID] partition-major -> [2, npc(padded), HID]
        z = np.asarray(zraw).reshape(2, P, nblk, HID).transpose(0, 2, 1, 3)
        return z.reshape(2, nblk * P, HID)

    if sim_mode:
        from concourse import bass_interp
        sim = bass_interp.MultiCoreSim(nc, CORES)
        for c in range(CORES):
            for k, v in in_maps[c].items():
                sim.cores[c].tensor(k)[:] = v
        sim.simulate()
        outs = [_unscramble(sim.cores[c].mem_tensor("zout").reshape(2, P, nblk * HID))
                for c in range(CORES)]
        z1 = np.concatenate([o[0] for o in outs], axis=0)[tables[0]["pos"]]
        z2 = np.concatenate([o[1] for o in outs], axis=0)[tables[1]["pos"]]
        return z1, z2

    kwargs = {}
    if trace:
        _install_profile_shim()
        kwargs["trace"] = True
    res = run_bass_kernel_spmd(nc, in_maps, core_ids=list(range(CORES)), **kwargs)
    LAST_EXEC_NS = res.exec_time_ns
    outs = [_unscramble(res.results[c]["zout"]) for c in range(CORES)]
    z1 = np.concatenate([o[0] for o in outs], axis=0)[tables[0]["pos"]]
    z2 = np.concatenate([o[1] for o in outs], axis=0)[tables[1]["pos"]]
    return z1, z2


def kernel(x1, edge_index1, x2, edge_index2, W1, b1, W2, b2):
    trace = bool(int(os.environ.get("KERNEL_TRACE", "0")))
    return _run(x1, edge_index1, x2, edge_index2, W1, b1, W2, b2,
                n_nodes=100000, trace=trace)



# revision 57
# speedup vs baseline: 1.0144x; 1.0144x over previous
"""CCA-SSG (2-layer GCN backbone x2 graphs + z-score) on 8 Trainium2 NeuronCores.

Strategy (graph/data parallel, per sharding hint):
  - Nodes row-sharded across 8 cores (12500/core). Edges routed to the core
    owning their destination. Weights replicated.
  - Algebraic restructure: with g = (x @ W) * dinv,  GCNConv output is
        out[d] = dinv[d] * (sum_{e: dst=d} g[src[e]] + g[d]) + b
    so the edge aggregation is an UNWEIGHTED segment-sum of gathered rows.
  - Per layer: compute local g shard -> AllGather full g table (HBM) ->
    dma_gather 256B rows by src -> one-hot matmul segment-sum into PSUM
    (S[e, j] = (dstloc[e] == j), agg = S^T @ G accumulated per 128-node block).
  - Gather table is split in 4 banks of <=25000 rows (dma_gather uses int16
    indices, read from SBUF partitions 16-31 on HW / 0-15 in CoreSim).
  - mean/std over nodes: per-core partial sum/sumsq via ones-matmul,
    AllReduce, broadcast back via K=1 matmul.

Host side does only sharding/routing work: edge bucketing by (bank, block),
padding, int16 index packing, x transpose-blocking, degree bincount.
"""
import math
import os
import sys

sys.path.insert(0, "/opt/trn_rl_repo")

import numpy as np

import concourse.bacc as bacc
import concourse.bass as bass
import concourse.mybir as mybir
import concourse.tile as tile
from concourse.bass_utils import run_bass_kernel_spmd

P = 128
CORES = 8
IN_DIM = 256
HID = 64  # = OUT_DIM; both layers have width 64
BANKS = 4
GCHUNK = 8    # chunks per dma_gather (num_idxs limit: >1024 crashes the Q7 ucode; re-verified)
SGROUP = 16   # chunks per is_equal S-build op

F32 = mybir.dt.float32
I16 = mybir.dt.int16

LAST_EXEC_NS = None


# ----------------------------------------------------------------------------
# host-side sharding / routing
# ----------------------------------------------------------------------------

def _balance(src, dst, n_nodes, npc, nblk, bank_rows):
    """Per-core within-core node permutation that balances per-(bank, block)
    in-degree sums, shrinking the shared bucket-max padding. Nodes stay on
    their original core (so self terms and bank membership are unchanged).

    Returns pos[n_nodes]: new global position of each node."""
    cores = n_nodes // npc
    q = src // bank_rows  # bank is invariant under within-core permutations
    v = np.zeros((n_nodes, BANKS), np.int64)
    np.add.at(v, (dst, q), 1)

    # wheel: positions (block, slot) in round-robin-over-blocks order, with
    # exact per-block capacities (last block may be short)
    sizes = np.full(nblk, P, np.int64)
    sizes[-1] = npc - (nblk - 1) * P
    wheel_pos = np.concatenate(
        [np.flatnonzero(sizes > j) * P + j for j in range(int(sizes.max()))])

    pos = np.empty(n_nodes, np.int64)
    for c in range(cores):
        vc = v[c * npc : (c + 1) * npc]
        _types, inv = np.unique(vc, axis=0, return_inverse=True)
        order = np.argsort(inv, kind="stable")  # nodes grouped by type
        p = np.empty(npc, np.int64)
        p[order] = wheel_pos
        blk = _swap_search(vc.astype(np.float64), p // P, sizes, npc, nblk)
        bo = np.argsort(blk, kind="stable")
        rank = np.arange(npc) - np.repeat(
            np.concatenate([[0], np.cumsum(np.bincount(blk, minlength=nblk))[:-1]]),
            np.bincount(blk, minlength=nblk))
        p2 = np.empty(npc, np.int64)
        p2[bo] = blk[bo] * P + rank
        pos[c * npc : (c + 1) * npc] = c * npc + p2
    return pos


def _swap_search(vc, blk, sizes, npc, nblk, rounds=25, K=30000, seed=0):
    """Pairwise-swap local search: minimize sum of squared per-(block, bank)
    deviations from size-proportional targets. Swaps preserve block sizes."""
    rng = np.random.default_rng(seed)
    tgt = sizes[:, None] * (vc.sum(axis=0)[None, :] / npc)

    def dev_of(b):
        s = np.zeros((nblk, vc.shape[1]))
        np.add.at(s, b, vc)
        return s - tgt

    dev = dev_of(blk)
    for _ in range(rounds):
        i = rng.integers(0, npc, K)
        j = rng.integers(0, npc, K)
        bi, bj = blk[i], blk[j]
        m = bi != bj
        i, j, bi, bj = i[m], j[m], bi[m], bj[m]
        delta = vc[j] - vc[i]
        dobj = (((dev[bi] + delta) ** 2).sum(1) + ((dev[bj] - delta) ** 2).sum(1)
                - (dev[bi] ** 2).sum(1) - (dev[bj] ** 2).sum(1))
        good = dobj < -1e-9
        i, j, bi, bj, dobj = i[good], j[good], bi[good], bj[good], dobj[good]
        o = np.argsort(dobj)
        i, j, bi, bj = i[o], j[o], bi[o], bj[o]
        used = np.zeros(nblk, bool)
        un = np.zeros(npc, bool)
        acc = []
        for k in range(len(i)):
            if used[bi[k]] or used[bj[k]] or un[i[k]] or un[j[k]]:
                continue
            used[bi[k]] = used[bj[k]] = True
            un[i[k]] = un[j[k]] = True
            acc.append(k)
        if not acc:
            break
        acc = np.asarray(acc, np.int64)
        ii, jj = i[acc], j[acc]
        blk[ii], blk[jj] = blk[jj], blk[ii]
        dev = dev_of(blk)
    return blk


def _route_graph(src, dst, n_nodes, npc, nblk, bank_rows):
    """Route edges by destination core; bucket by (bank(src), block(dst)).

    Packed layout: within each bank, bucket segments are laid out back to back
    (segment length = max bucket count over cores), so chunks may straddle
    bucket boundaries. A straddling chunk gets one "sentry" (S-matrix column
    group + matmul) per bucket it overlaps; out-of-bucket rows are poisoned
    with dstloc=255 so the one-hot never matches them.

    Returns dict(totch, specs, sentries) + per-core (idx_stream, dstl [nsent,P])."""
    cores = n_nodes // npc
    per_core = []
    counts = np.zeros((cores, BANKS, nblk), np.int64)
    for c in range(cores):
        lo, hi = c * npc, (c + 1) * npc
        m = (dst >= lo) & (dst < hi)
        s = src[m]
        dl = (dst[m] - lo).astype(np.int64)
        blk = dl >> 7
        q = s // bank_rows
        order = np.lexsort((s, blk, q))
        s, dl, blk, q = s[order], dl[order], blk[order], q[order]
        np.add.at(counts[c], (q, blk), 1)
        per_core.append((s, dl))

    seg = counts.max(axis=0)  # [BANKS, nblk] shared bucket segment lengths
    bank_nch = [int(-(-int(seg[q].sum()) // P)) for q in range(BANKS)]
    bank_c0 = np.concatenate([[0], np.cumsum(bank_nch)]).astype(np.int64)
    totch = int(bank_c0[-1])
    bucket_off = np.zeros((BANKS, nblk), np.int64)
    for q in range(BANKS):
        bucket_off[q] = bank_c0[q] * P + np.concatenate(
            [[0], np.cumsum(seg[q])[:-1]])

    # sentries: (chunk, block, is_first_of_run, is_last_of_run)
    sentries = []
    for q in range(BANKS):
        for b in range(nblk):
            o, n = int(bucket_off[q, b]), int(seg[q, b])
            if n == 0:
                continue
            c_first, c_last = o // P, (o + n - 1) // P
            for ci in range(c_first, c_last + 1):
                sentries.append((ci, b, ci == c_first, ci == c_last))
    nsent = len(sentries)

    # gather specs: per bank, chunks in groups of <= GCHUNK
    specs = []
    for q in range(BANKS):
        done = 0
        while done < bank_nch[q]:
            k = min(GCHUNK, bank_nch[q] - done)
            specs.append((q, int(bank_c0[q]) + done, k))
            done += k

    out = []
    for c in range(cores):
        s, dl = per_core[c]
        cnt = counts[c]
        idx_stream = np.zeros(totch * P, np.int16)
        dst_pos = np.full(totch * P, 255.0, np.float32)
        pos_in = 0
        for q in range(BANKS):
            for b in range(nblk):
                n = int(cnt[q, b])
                if n == 0:
                    continue
                o = int(bucket_off[q, b])
                idx_stream[o : o + n] = (
                    s[pos_in : pos_in + n] - q * bank_rows).astype(np.int16)
                dst_pos[o : o + n] = (
                    dl[pos_in : pos_in + n] - b * P).astype(np.float32)
                pos_in += n
        assert pos_in == len(s)
        # per-sentry dstloc columns, masked to the sentry's bucket span
        dstl = np.full((nsent, P), 255.0, np.float32)
        for j, (ci, b, _st, _sp) in enumerate(sentries):
            q = int(np.searchsorted(bank_c0, ci, side="right")) - 1
            o, n = int(bucket_off[q, b]), int(seg[q, b])
            lo_p = max(o, ci * P)
            hi_p = min(o + n, (ci + 1) * P)
            dstl[j, lo_p - ci * P : hi_p - ci * P] = dst_pos[lo_p:hi_p]
        out.append((idx_stream, dstl))
    return {"totch": totch, "specs": specs, "sentries": sentries}, out


def _pack_idx16(idx_stream, specs):
    """[128, totch*8] int16: per gather instruction local index j lives at
    row 16 + j%16 (HW) and j%16 (CoreSim), column c0*8 + j//16."""
    totch = len(idx_stream) // P
    arr = np.zeros((P, totch * 8), np.int16)
    for (_q, c0, nch) in specs:
        seg = idx_stream[c0 * P : (c0 + nch) * P]
        w = seg.reshape(-1, 16).T  # [16, nch*8]
        arr[0:16, c0 * 8 : (c0 + nch) * 8] = w
        arr[16:32, c0 * 8 : (c0 + nch) * 8] = w
    return arr


# ----------------------------------------------------------------------------
# device kernel builder
# ----------------------------------------------------------------------------

def _build_nc(n_nodes, npc, nblk, bank_rows, tables, split=True):
    """tables: per graph dict(totch, specs, sentries)"""
    npc_pad = nblk * P
    last_rows = npc - (nblk - 1) * P

    nc = bacc.Bacc(None, target_bir_lowering=False, debug=False)

    # ---- parameters (per core) ----
    XB = 7  # x-tile DMA batch (blocks per load); nblk padded to a multiple
    ngrp = -(-nblk // XB)
    xtb = [nc.declare_dram_parameter(f"xtb{g}", [ngrp, P, 2 * XB * P], F32, isOutput=False)
           for g in range(2)]
    deg_in = [nc.declare_dram_parameter(f"deg{g}", [P, nblk], F32, isOutput=False)
              for g in range(2)]
    dstl_in = [nc.declare_dram_parameter(f"dstloc{g}", [P, len(tables[g]["sentries"])], F32, isOutput=False)
               for g in range(2)]
    idx_in = [nc.declare_dram_parameter(f"idx{g}", [P, tables[g]["totch"] * 8], I16, isOutput=False)
              for g in range(2)]
    w1p_in = nc.declare_dram_parameter("w1p", [P, 2 * HID], F32, isOutput=False)
    w2_in = nc.declare_dram_parameter("w2", [HID, HID], F32, isOutput=False)
    b1_in = nc.declare_dram_parameter("b1t", [P, HID], F32, isOutput=False)
    b2_in = nc.declare_dram_parameter("b2t", [P, HID], F32, isOutput=False)
    iota_in = nc.declare_dram_parameter("iota", [P, P], F32, isOutput=False)
    ident_in = nc.declare_dram_parameter("ident", [P, P], F32, isOutput=False)
    ones_in = nc.declare_dram_parameter("ones", [P, P], F32, isOutput=False)
    zout = nc.declare_dram_parameter("zout", [2, P, nblk * HID], F32, isOutput=True)

    # ---- internal DRAM ----
    g_shard = [[nc.dram_tensor(f"gshard{g}_{l}", [npc, HID], F32) for l in range(2)]
               for g in range(2)]
    g_full = [[nc.dram_tensor(f"gfull{g}_{l}", [n_nodes, HID], F32, addr_space="Shared")
               for l in range(2)] for g in range(2)]
    g_mir = [[nc.dram_tensor(f"gmir{g}_{l}", [n_nodes, HID], F32) for l in range(2)]
             for g in range(2)]
    stats_in = nc.dram_tensor("stats_in", [1, 4 * HID], F32)
    stats_out = nc.dram_tensor("stats_out", [1, 4 * HID], F32, addr_space="Shared")

    rg = [list(range(CORES))]

    with tile.TileContext(nc) as tc:
        with (
            tc.tile_pool(name="const", bufs=1) as cpool,
            tc.tile_pool(name="acc", bufs=1) as apool,
            tc.tile_pool(name="work", bufs=3) as wpool,
            tc.tile_pool(name="blk", bufs=4) as bpool,
            tc.tile_pool(name="psA", bufs=2, space="PSUM") as psA,
            tc.tile_pool(name="psTr", bufs=1, space="PSUM") as psTr,
            tc.tile_pool(name="psAgg", bufs=3, space="PSUM") as psAgg,
            tc.tile_pool(name="psSm", bufs=1, space="PSUM") as psSm,
        ):
            # ---- constants ----
            w1p = cpool.tile([P, 2 * HID], F32)
            nc.sync.dma_start(w1p[:], w1p_in[:])
            w2sb = cpool.tile([HID, HID], F32)
            nc.sync.dma_start(w2sb[:], w2_in[:])
            b1sb = cpool.tile([P, HID], F32)
            nc.sync.dma_start(b1sb[:], b1_in[:])
            b2sb = cpool.tile([P, HID], F32)
            nc.sync.dma_start(b2sb[:], b2_in[:])
            iota = cpool.tile([P, P], F32)
            nc.sync.dma_start(iota[:], iota_in[:])
            ident = cpool.tile([P, P], F32)
            nc.sync.dma_start(ident[:], ident_in[:])
            ones = cpool.tile([P, P], F32)
            nc.sync.dma_start(ones[:], ones_in[:])
            ones_col = ones[:, 0:1]         # [128, 1] of ones
            ones_row = ones[0:1, :]         # [1, 128] of ones

            dinv = []
            for g in range(2):
                dt = cpool.tile([P, nblk], F32, tag=f"deg{g}")
                nc.sync.dma_start(dt[:], deg_in[g][:])
                sq = cpool.tile([P, nblk], F32, tag=f"dsq{g}")
                nc.scalar.activation(sq[:], dt[:], mybir.ActivationFunctionType.Sqrt)
                dv = cpool.tile([P, nblk], F32, tag=f"dinv{g}")
                nc.vector.reciprocal(dv[:], sq[:])
                dinv.append(dv)

            accB = [apool.tile([P, nblk * HID], F32, tag=f"accB{g}", name=f"accB{g}") for g in range(2)]
            accC = [apool.tile([P, nblk * HID], F32, tag=f"accC{g}", name=f"accC{g}") for g in range(2)]

            def rows_of(b):
                return last_rows if b == nblk - 1 else P

            # ---- phase A: g0 = (x @ W1) * dinv, allgather ----
            for g in range(2):
                for b in range(nblk):
                    if b % XB == 0:
                        xt4 = wpool.tile([P, 2 * XB * P], F32, tag="xt", bufs=2)
                        nc.sync.dma_start(xt4[:], xtb[g][b // XB])
                    ph = psA.tile([P, HID], F32, tag="hps")
                    for k in range(2):
                        ko = ((b % XB) * 2 + k) * P
                        xt = xt4[:, ko : ko + P]
                        nc.tensor.matmul(
                            out=ph[:], lhsT=xt, rhs=w1p[:, k * HID : (k + 1) * HID],
                            start=(k == 0), stop=(k == 1))
                    gblk = accB[g][:, b * HID : (b + 1) * HID]
                    nc.scalar.activation(gblk, ph[:],
                                         mybir.ActivationFunctionType.Copy,
                                         scale=dinv[g][:, b : b + 1])
                    r = rows_of(b)
                    nc.sync.dma_start(g_shard[g][0][b * P : b * P + r, :], accB[g][:r, b * HID : (b + 1) * HID])
                nc.gpsimd.collective_compute(
                    "AllGather", mybir.AluOpType.bypass, replica_groups=rg,
                    ins=[g_shard[g][0][:]], outs=[g_full[g][0][:]])

            # ---- aggregation emitter ----
            dstl_tiles = {}
            for g in range(2):
                dt_ = cpool.tile([P, len(tables[g]["sentries"])], F32, tag=f"dstl{g}")
                nc.sync.dma_start(dt_[:], dstl_in[g][:])
                dstl_tiles[g] = dt_

            def aggregate(g, layer, acc):
                """acc[:, b*64:(b+1)*64] += segment_sum of gathered g rows."""
                if int(os.environ.get("KERNEL_NO_AGG", "0")):
                    return
                t = tables[g]
                specs, sentries = t["specs"], t["sentries"]
                dstl = dstl_tiles[g]
                table = g_full[g][layer]
                gt = {}
                spec_i = 0
                stile = None
                sbase = 0
                ps = None
                nsent = len(sentries)
                for j, (ci, b, st, sp) in enumerate(sentries):
                    # emit every gather whose chunk range we have now reached
                    while spec_i < len(specs) and specs[spec_i][1] <= ci:
                        q, c0, nch = specs[spec_i]
                        it = wpool.tile([P, GCHUNK * 8], I16, tag="idx", bufs=6)
                        nc.sync.dma_start(it[:, : nch * 8], idx_in[g][:, c0 * 8 : (c0 + nch) * 8])
                        gtile = wpool.tile([P, GCHUNK * HID], F32, tag="gt", bufs=6)
                        nc.gpsimd.dma_gather(
                            gtile[:, : nch * HID].rearrange("p (c d) -> p c d", c=nch),
                            table[q * bank_rows : (q + 1) * bank_rows, :],
                            it[:, : nch * 8], nch * P, nch * P, HID)
                        gt = {"tile": gtile, "c0": c0}
                        spec_i += 1
                    if j % SGROUP == 0:
                        ns = min(SGROUP, nsent - j)
                        stile = wpool.tile([P, SGROUP * P], F32, tag="stile")
                        s3 = stile[:, : ns * P].rearrange("p (c j) -> p c j", c=ns)
                        nc.vector.tensor_tensor(
                            out=s3,
                            in0=dstl[:, j : j + ns][:, :, None].to_broadcast([P, ns, P]),
                            in1=iota[:, None, :].to_broadcast([P, ns, P]),
                            op=mybir.AluOpType.is_equal)
                        sbase = j
                    if st:
                        ps = psAgg.tile([P, HID], F32, tag="aggps")
                    co = ci - gt["c0"]
                    nc.tensor.matmul(
                        out=ps[:],
                        lhsT=stile[:, (j - sbase) * P : (j - sbase + 1) * P],
                        rhs=gt["tile"][:, co * HID : (co + 1) * HID],
                        start=st, stop=sp, skip_group_check=True)
                    if sp:
                        sl = acc[:, b * HID : (b + 1) * HID]
                        nc.vector.tensor_tensor(out=sl, in0=sl, in1=ps[:],
                                                op=mybir.AluOpType.add)

            # ---- phase B: layer-1 aggregation, relu, @W2, allgather ----
            for g in range(2):
                aggregate(g, 0, accB[g])
                for b in range(nblk):
                    sl = accB[g][:, b * HID : (b + 1) * HID]
                    t1 = bpool.tile([P, HID], F32, tag="t1")
                    nc.scalar.activation(t1[:], sl, mybir.ActivationFunctionType.Copy,
                                         scale=dinv[g][:, b : b + 1])
                    t2 = bpool.tile([P, HID], F32, tag="t2")
                    nc.vector.tensor_tensor(out=t2[:], in0=t1[:], in1=b1sb[:],
                                            op=mybir.AluOpType.add)
                    r = bpool.tile([P, HID], F32, tag="t3")
                    nc.scalar.activation(r[:], t2[:], mybir.ActivationFunctionType.Relu)
                    trp = psTr.tile([HID, P], F32, tag="trps")
                    nc.tensor.transpose(out=trp[:], in_=r[:], identity=ident[:])
                    trs = bpool.tile([HID, P], F32, tag="trs")
                    nc.vector.tensor_copy(trs[:], trp[:])
                    p2 = psA.tile([P, HID], F32, tag="hps")
                    nc.tensor.matmul(out=p2[:], lhsT=trs[:], rhs=w2sb[:],
                                     start=True, stop=True)
                    g2b = accC[g][:, b * HID : (b + 1) * HID]
                    nc.scalar.activation(g2b, p2[:], mybir.ActivationFunctionType.Copy,
                                         scale=dinv[g][:, b : b + 1])
                    rr = rows_of(b)
                    nc.sync.dma_start(g_shard[g][1][b * P : b * P + rr, :], accC[g][:rr, b * HID : (b + 1) * HID])
                nc.gpsimd.collective_compute(
                    "AllGather", mybir.AluOpType.bypass, replica_groups=rg,
                    ins=[g_shard[g][1][:]], outs=[g_full[g][1][:]])

            # ---- phase C: layer-2 aggregation, out2, stats ----
            stats_sb = cpool.tile([1, 4 * HID], F32, tag="stats_sb")
            for g in range(2):
                aggregate(g, 1, accC[g])
                pst = psSm.tile([1, 2 * HID], F32, tag="pstats", name="pst")
                for b in range(nblk):
                    sl = accC[g][:, b * HID : (b + 1) * HID]
                    t1 = bpool.tile([P, HID], F32, tag="t1")
                    nc.scalar.activation(t1[:], sl, mybir.ActivationFunctionType.Copy,
                                         scale=dinv[g][:, b : b + 1])
                    o2sq = bpool.tile([P, 2 * HID], F32, tag="t2")
                    o2 = o2sq[:, :HID]
                    sq = o2sq[:, HID:]
                    nc.vector.tensor_tensor(out=o2, in0=t1[:], in1=b2sb[:],
                                            op=mybir.AluOpType.add)
                    nc.vector.tensor_tensor(out=sq, in0=o2, in1=o2,
                                            op=mybir.AluOpType.mult)
                    # overwrite accC block in place with the final conv2 output
                    nc.scalar.activation(sl, o2, mybir.ActivationFunctionType.Copy)
                    rr = rows_of(b)
                    nc.tensor.matmul(out=pst[:], lhsT=ones_col[:rr], rhs=o2sq[:rr, :],
                                     start=(b == 0), stop=(b == nblk - 1),
                                     skip_group_check=True)
                nc.vector.tensor_copy(stats_sb[:, 2 * HID * g : 2 * HID * (g + 1)], pst[:])
            nc.sync.dma_start(stats_in[:], stats_sb[:])
            nc.gpsimd.collective_compute(
                "AllReduce", mybir.AluOpType.add, replica_groups=rg,
                ins=[stats_in[:]], outs=[stats_out[:]])
            stats_rx = cpool.tile([1, 4 * HID], F32, tag="stats_rx")
            nc.sync.dma_start(stats_rx[:], stats_out[:])

            # ---- z-score ----
            n_f = float(n_nodes)
            for g in range(2):
                srow = stats_rx[:, 2 * HID * g : 2 * HID * g + HID]
                qrow = stats_rx[:, 2 * HID * g + HID : 2 * HID * (g + 1)]
                mean = cpool.tile([1, HID], F32, tag=f"mean{g}")
                nc.scalar.activation(mean[:], srow, mybir.ActivationFunctionType.Copy,
                                     scale=1.0 / n_f)
                s2 = cpool.tile([1, HID], F32, tag=f"s2_{g}")
                nc.vector.tensor_tensor(out=s2[:], in0=srow, in1=srow,
                                        op=mybir.AluOpType.mult)
                s2n = cpool.tile([1, HID], F32, tag=f"s2n{g}")
                nc.scalar.activation(s2n[:], s2[:], mybir.ActivationFunctionType.Copy,
                                     scale=1.0 / n_f)
                v = cpool.tile([1, HID], F32, tag=f"v{g}")
                nc.vector.tensor_tensor(out=v[:], in0=qrow, in1=s2n[:],
                                        op=mybir.AluOpType.subtract)
                stdv = cpool.tile([1, HID], F32, tag=f"std{g}")
                nc.scalar.activation(stdv[:], v[:], mybir.ActivationFunctionType.Sqrt,
                                     scale=1.0 / (n_f - 1.0))
                rstd = cpool.tile([1, HID], F32, tag=f"rstd{g}")
                nc.vector.reciprocal(rstd[:], stdv[:])
                pb = psSm.tile([P, 2 * HID], F32, tag="bcast")
                pm = pb[:, :HID]
                pr = pb[:, HID:]
                nc.tensor.matmul(out=pm, lhsT=ones_row, rhs=mean[:],
                                 start=True, stop=True, skip_group_check=True)
                nc.tensor.matmul(out=pr, lhsT=ones_row, rhs=rstd[:],
                                 start=True, stop=True, skip_group_check=True)
                ZB = 14  # blocks per z-score slab (98 = 7*14)
                for s in range(0, nblk, ZB):
                    nb2 = min(ZB, nblk - s)
                    slab = accC[g][:, s * HID : (s + nb2) * HID]
                    s3 = slab.rearrange("p (c f) -> p c f", c=nb2)
                    z1 = bpool.tile([P, ZB * HID], F32, tag="z1", bufs=2)
                    z13 = z1[:, : nb2 * HID].rearrange("p (c f) -> p c f", c=nb2)
                    nc.vector.tensor_tensor(
                        out=z13, in0=s3,
                        in1=pm[:, None, :].to_broadcast([P, nb2, HID]),
                        op=mybir.AluOpType.subtract)
                    z2 = bpool.tile([P, ZB * HID], F32, tag="z2", bufs=2)
                    z23 = z2[:, : nb2 * HID].rearrange("p (c f) -> p c f", c=nb2)
                    nc.vector.tensor_tensor(
                        out=z23, in0=z13,
                        in1=pr[:, None, :].to_broadcast([P, nb2, HID]),
                        op=mybir.AluOpType.mult)
                    nc.sync.dma_start(
                        zout[g, :, s * HID : (s + nb2) * HID],
                        z2[:, : nb2 * HID])

    nc.compile()
    if split:
        _split_waits(nc, max_waits=1)
    return nc


# ----------------------------------------------------------------------------
# wait-splitting post-pass (walrus rejects >1 sync wait per instruction here)
# ----------------------------------------------------------------------------

def _split_waits(nc, max_waits=1):
    inserted = 0
    for blk in nc.main_func.blocks:
        bb = blk if hasattr(blk, "instructions") else blk.bb
        new_list = []
        for ins in bb.instructions:
            si = ins.sync_info
            waits = list(si.on_wait) if (si and si.on_wait) else []
            if len(waits) > max_waits:
                keep = waits[-max_waits:]
                extra = waits[:-max_waits]
                for i in range(0, len(extra), max_waits):
                    chunk = extra[i : i + max_waits]
                    nop = mybir.InstNoOp(
                        name=nc.get_next_instruction_name(),
                        engine=ins.engine, ins=[], outs=[], text_hint="wait_split")
                    nop.sync_info = mybir.SyncInfo(on_wait=chunk, on_update=[])
                    new_list.append(nop)
                    inserted += 1
                si.on_wait = keep
            new_list.append(ins)
        bb.instructions[:] = new_list
    return inserted


# ----------------------------------------------------------------------------
# host wrapper
# ----------------------------------------------------------------------------

def _prepare(x1, edge_index1, x2, edge_index2, W1, b1, W2, b2, n_nodes):
    npc = n_nodes // CORES
    nblk = -(-npc // P)
    npc_pad = nblk * P
    bank_rows = -(-n_nodes // BANKS)
    assert bank_rows <= 32767

    graphs = [(np.asarray(x1), np.asarray(edge_index1)),
              (np.asarray(x2), np.asarray(edge_index2))]
    tables = []
    per_core_arrays = [dict() for _ in range(CORES)]
    for g, (x, ei) in enumerate(graphs):
        src0 = np.asarray(ei[0], dtype=np.int64)
        dst0 = np.asarray(ei[1], dtype=np.int64)
        pos = _balance(src0, dst0, n_nodes, npc, nblk, bank_rows)
        inv = np.empty(n_nodes, np.int64)
        inv[pos] = np.arange(n_nodes)
        src = pos[src0]
        dst = pos[dst0]
        deg = np.bincount(dst, minlength=n_nodes).astype(np.float32) + 1.0
        table, routed = _route_graph(src, dst, n_nodes, npc, nblk, bank_rows)
        table["pos"] = pos
        tables.append(table)
        x = np.asarray(x, dtype=np.float32)
        for c in range(CORES):
            idx_stream, dstl = routed[c]
            d = per_core_arrays[c]
            XB = 7
            ngrp = -(-nblk // XB)
            xp = np.zeros((ngrp * XB * P, IN_DIM), np.float32)
            xp[:npc] = x[inv[c * npc : (c + 1) * npc]]
            # [grp, q, b_in_grp, k, p] so each group is one 2D SBUF-layout DMA
            d[f"xtb{g}"] = np.ascontiguousarray(
                xp.reshape(ngrp, XB, P, 2, P).transpose(0, 4, 1, 3, 2)
                .reshape(ngrp, P, 2 * XB * P))
            degp = np.ones(npc_pad, np.float32)
            degp[:npc] = deg[c * npc : (c + 1) * npc]
            d[f"deg{g}"] = np.ascontiguousarray(degp.reshape(nblk, P).T)
            d[f"dstloc{g}"] = np.ascontiguousarray(dstl.T)
            d[f"idx{g}"] = _pack_idx16(idx_stream, table["specs"])

    W1 = np.asarray(W1, np.float32)
    w1p = np.zeros((P, 2 * HID), np.float32)
    w1p[:, :HID] = W1[:P]
    w1p[:, HID:] = W1[P:]
    shared = {
        "w1p": w1p,
        "w2": np.asarray(W2, np.float32),
        "b1t": np.broadcast_to(np.asarray(b1, np.float32), (P, HID)).copy(),
        "b2t": np.broadcast_to(np.asarray(b2, np.float32), (P, HID)).copy(),
        "iota": np.tile(np.arange(P, dtype=np.float32), (P, 1)),
        "ident": np.eye(P, dtype=np.float32),
        "ones": np.ones((P, P), np.float32),
    }
    for d in per_core_arrays:
        d.update(shared)
    return tables, per_core_arrays, npc, nblk, bank_rows



def _install_profile_shim():
    """ctypes NTFF hook for run_bass_kernel_spmd(trace=True) under axon."""
    import contextlib
    import ctypes
    import types
    if "antenv.axon_hooks" in sys.modules:
        return
    try:
        lib = ctypes.CDLL("/opt/axon/libaxon_pjrt.so")
        lib.axon_start_nrt_profile.argtypes = [ctypes.POINTER(ctypes.c_int64), ctypes.c_size_t]
        lib.axon_start_nrt_profile.restype = ctypes.c_int64
        lib.axon_stop_nrt_profile.argtypes = [ctypes.c_char_p]
        lib.axon_stop_nrt_profile.restype = ctypes.c_int64
    except (OSError, AttributeError):
        return

    @contextlib.contextmanager
    def _hook(output_dir, device_ids):
        import jax
        jax.devices()
        if device_ids:
            ids = (ctypes.c_int64 * len(device_ids))(*device_ids)
            rc = lib.axon_start_nrt_profile(ids, len(device_ids))
        else:
            rc = lib.axon_start_nrt_profile(None, 0)
        if rc != 0:
            raise RuntimeError(f"axon_start_nrt_profile rc={rc}")
        try:
            yield
        finally:
            n = lib.axon_stop_nrt_profile(str(output_dir).encode())
            print(f"ntff profile: {n} file(s) -> {output_dir}", file=sys.stderr)

    mod = types.ModuleType("antenv.axon_hooks")
    mod.get_axon_ntff_profile_hook = lambda: _hook
    mod.set_axon_ntff_profile_hook = lambda h: None
    sys.modules["antenv.axon_hooks"] = mod

    from concourse import bass_utils
    bass_utils.upload_artifacts = lambda tmpdir: f"local:{tmpdir}"

_NC_CACHE = {}


def _run(x1, edge_index1, x2, edge_index2, W1, b1, W2, b2, n_nodes, trace=False):
    global LAST_EXEC_NS
    tables, in_maps, npc, nblk, bank_rows = _prepare(
        x1, edge_index1, x2, edge_index2, W1, b1, W2, b2, n_nodes)

    sim_mode = bool(int(os.environ.get("KERNEL_SIM", "0")))
    key = (n_nodes, sim_mode,
           tuple(tables[0]["sentries"]), tuple(tables[0]["specs"]),
           tuple(tables[1]["sentries"]), tuple(tables[1]["specs"]))
    if key not in _NC_CACHE:
        _NC_CACHE[key] = _build_nc(n_nodes, npc, nblk, bank_rows, tables,
                                   split=not sim_mode)
    nc = _NC_CACHE[key]

    def _unscramble(zraw):
        # [2, P, nblk*HID] partition-major -> [2, npc, HID]
        z = np.asarray(zraw).reshape(2, P, nblk, HID).transpose(0, 2, 1, 3)
        return z.reshape(2, nblk * P, HID)[:, :npc]

    if sim_mode:
        from concourse import bass_interp
        sim = bass_interp.MultiCoreSim(nc, CORES)
        for c in range(CORES):
            for k, v in in_maps[c].items():
                sim.cores[c].tensor(k)[:] = v
        sim.simulate()
        outs = [_unscramble(sim.cores[c].mem_tensor("zout").reshape(2, P, nblk * HID))
                for c in range(CORES)]
        z1 = np.concatenate([o[0] for o in outs], axis=0)[tables[0]["pos"]]
        z2 = np.concatenate([o[1] for o in outs], axis=0)[tables[1]["pos"]]
        return z1, z2

    kwargs = {}
    if trace:
        _install_profile_shim()
        kwargs["trace"] = True
    res = run_bass_kernel_spmd(nc, in_maps, core_ids=list(range(CORES)), **kwargs)
    LAST_EXEC_NS = res.exec_time_ns
    outs = [_unscramble(res.results[c]["zout"]) for c in range(CORES)]
    z1 = np.concatenate([o[0] for o in outs], axis=0)[tables[0]["pos"]]
    z2 = np.concatenate([o[1] for o in outs], axis=0)[tables[1]["pos"]]
    return z1, z2


def kernel(x1, edge_index1, x2, edge_index2, W1, b1, W2, b2):
    trace = bool(int(os.environ.get("KERNEL_TRACE", "0")))
    return _run(x1, edge_index1, x2, edge_index2, W1, b1, W2, b2,
                n_nodes=100000, trace=trace)



# revision 64
# speedup vs baseline: 1.0365x; 1.0217x over previous
"""CCA-SSG (2-layer GCN backbone x2 graphs + z-score) on 8 Trainium2 NeuronCores.

Strategy (graph/data parallel, per sharding hint):
  - Nodes row-sharded across 8 cores (12500/core). Edges routed to the core
    owning their destination. Weights replicated.
  - Algebraic restructure: with g = (x @ W) * dinv,  GCNConv output is
        out[d] = dinv[d] * (sum_{e: dst=d} g[src[e]] + g[d]) + b
    so the edge aggregation is an UNWEIGHTED segment-sum of gathered rows.
  - Per layer: compute local g shard -> AllGather full g table (HBM) ->
    dma_gather 256B rows by src -> one-hot matmul segment-sum into PSUM
    (S[e, j] = (dstloc[e] == j), agg = S^T @ G accumulated per 128-node block).
  - Gather table is split in 4 banks of <=25000 rows (dma_gather uses int16
    indices, read from SBUF partitions 16-31 on HW / 0-15 in CoreSim).
  - mean/std over nodes: per-core partial sum/sumsq via ones-matmul,
    AllReduce, broadcast back via K=1 matmul.

Host side does only sharding/routing work: edge bucketing by (bank, block),
padding, int16 index packing, x transpose-blocking, degree bincount.
"""
import math
import os
import sys

sys.path.insert(0, "/opt/trn_rl_repo")

import numpy as np

import concourse.bacc as bacc
import concourse.bass as bass
import concourse.mybir as mybir
import concourse.tile as tile
from concourse.bass_utils import run_bass_kernel_spmd

P = 128
CORES = 8
IN_DIM = 256
HID = 64  # = OUT_DIM; both layers have width 64
BANKS = 4
GCHUNK = 8    # chunks per dma_gather (num_idxs limit: >1024 crashes the Q7 ucode; re-verified)
SGROUP = 16   # chunks per is_equal S-build op

F32 = mybir.dt.float32
I16 = mybir.dt.int16

LAST_EXEC_NS = None


# ----------------------------------------------------------------------------
# host-side sharding / routing
# ----------------------------------------------------------------------------

def _balance(src, dst, n_nodes, npc, nblk, bank_rows):
    """Per-core within-core node permutation that balances per-(bank, block)
    in-degree sums, shrinking the shared bucket-max padding. Nodes stay on
    their original core (so self terms and bank membership are unchanged).

    Returns pos[n_nodes]: new global position of each node."""
    cores = n_nodes // npc
    q = src // bank_rows  # bank is invariant under within-core permutations
    v = np.zeros((n_nodes, BANKS), np.int64)
    np.add.at(v, (dst, q), 1)

    # wheel: positions (block, slot) in round-robin-over-blocks order, with
    # exact per-block capacities (last block may be short)
    sizes = np.full(nblk, P, np.int64)
    sizes[-1] = npc - (nblk - 1) * P
    wheel_pos = np.concatenate(
        [np.flatnonzero(sizes > j) * P + j for j in range(int(sizes.max()))])

    pos = np.empty(n_nodes, np.int64)
    for c in range(cores):
        vc = v[c * npc : (c + 1) * npc]
        _types, inv = np.unique(vc, axis=0, return_inverse=True)
        order = np.argsort(inv, kind="stable")  # nodes grouped by type
        p = np.empty(npc, np.int64)
        p[order] = wheel_pos
        blk = _swap_search(vc.astype(np.float64), p // P, sizes, npc, nblk)
        bo = np.argsort(blk, kind="stable")
        rank = np.arange(npc) - np.repeat(
            np.concatenate([[0], np.cumsum(np.bincount(blk, minlength=nblk))[:-1]]),
            np.bincount(blk, minlength=nblk))
        p2 = np.empty(npc, np.int64)
        p2[bo] = blk[bo] * P + rank
        pos[c * npc : (c + 1) * npc] = c * npc + p2
    return pos


def _swap_search(vc, blk, sizes, npc, nblk, rounds=25, K=30000, seed=0):
    """Pairwise-swap local search: minimize sum of squared per-(block, bank)
    deviations from size-proportional targets. Swaps preserve block sizes."""
    rng = np.random.default_rng(seed)
    tgt = sizes[:, None] * (vc.sum(axis=0)[None, :] / npc)

    def dev_of(b):
        s = np.zeros((nblk, vc.shape[1]))
        np.add.at(s, b, vc)
        return s - tgt

    dev = dev_of(blk)
    for _ in range(rounds):
        i = rng.integers(0, npc, K)
        j = rng.integers(0, npc, K)
        bi, bj = blk[i], blk[j]
        m = bi != bj
        i, j, bi, bj = i[m], j[m], bi[m], bj[m]
        delta = vc[j] - vc[i]
        dobj = (((dev[bi] + delta) ** 2).sum(1) + ((dev[bj] - delta) ** 2).sum(1)
                - (dev[bi] ** 2).sum(1) - (dev[bj] ** 2).sum(1))
        good = dobj < -1e-9
        i, j, bi, bj, dobj = i[good], j[good], bi[good], bj[good], dobj[good]
        o = np.argsort(dobj)
        i, j, bi, bj = i[o], j[o], bi[o], bj[o]
        used = np.zeros(nblk, bool)
        un = np.zeros(npc, bool)
        acc = []
        for k in range(len(i)):
            if used[bi[k]] or used[bj[k]] or un[i[k]] or un[j[k]]:
                continue
            used[bi[k]] = used[bj[k]] = True
            un[i[k]] = un[j[k]] = True
            acc.append(k)
        if not acc:
            break
        acc = np.asarray(acc, np.int64)
        ii, jj = i[acc], j[acc]
        blk[ii], blk[jj] = blk[jj], blk[ii]
        dev = dev_of(blk)
    return blk


def _route_graph(src, dst, n_nodes, npc, nblk, bank_rows):
    """Route edges by destination core; bucket by (bank(src), block(dst)).

    Packed layout: within each bank, bucket segments are laid out back to back
    (segment length = max bucket count over cores), so chunks may straddle
    bucket boundaries. A straddling chunk gets one "sentry" (S-matrix column
    group + matmul) per bucket it overlaps; out-of-bucket rows are poisoned
    with dstloc=255 so the one-hot never matches them.

    Returns dict(totch, specs, sentries) + per-core (idx_stream, dstl [nsent,P])."""
    cores = n_nodes // npc
    per_core = []
    counts = np.zeros((cores, BANKS, nblk), np.int64)
    for c in range(cores):
        lo, hi = c * npc, (c + 1) * npc
        m = (dst >= lo) & (dst < hi)
        s = src[m]
        dl = (dst[m] - lo).astype(np.int64)
        blk = dl >> 7
        q = s // bank_rows
        order = np.lexsort((s, blk, q))
        s, dl, blk, q = s[order], dl[order], blk[order], q[order]
        np.add.at(counts[c], (q, blk), 1)
        per_core.append((s, dl))

    seg = counts.max(axis=0)  # [BANKS, nblk] shared bucket segment lengths
    bank_nch = [int(-(-int(seg[q].sum()) // P)) for q in range(BANKS)]
    bank_c0 = np.concatenate([[0], np.cumsum(bank_nch)]).astype(np.int64)
    totch = int(bank_c0[-1])
    bucket_off = np.zeros((BANKS, nblk), np.int64)
    for q in range(BANKS):
        bucket_off[q] = bank_c0[q] * P + np.concatenate(
            [[0], np.cumsum(seg[q])[:-1]])

    # sentries: (chunk, block, is_first_of_run, is_last_of_run)
    sentries = []
    for q in range(BANKS):
        for b in range(nblk):
            o, n = int(bucket_off[q, b]), int(seg[q, b])
            if n == 0:
                continue
            c_first, c_last = o // P, (o + n - 1) // P
            for ci in range(c_first, c_last + 1):
                sentries.append((ci, b, ci == c_first, ci == c_last))
    nsent = len(sentries)

    # gather specs: per bank, chunks in groups of <= GCHUNK
    specs = []
    for q in range(BANKS):
        done = 0
        while done < bank_nch[q]:
            k = min(GCHUNK, bank_nch[q] - done)
            specs.append((q, int(bank_c0[q]) + done, k))
            done += k

    out = []
    for c in range(cores):
        s, dl = per_core[c]
        cnt = counts[c]
        idx_stream = np.zeros(totch * P, np.int16)
        dst_pos = np.full(totch * P, 255.0, np.float32)
        pos_in = 0
        for q in range(BANKS):
            for b in range(nblk):
                n = int(cnt[q, b])
                if n == 0:
                    continue
                o = int(bucket_off[q, b])
                idx_stream[o : o + n] = (
                    s[pos_in : pos_in + n] - q * bank_rows).astype(np.int16)
                dst_pos[o : o + n] = (
                    dl[pos_in : pos_in + n] - b * P).astype(np.float32)
                pos_in += n
        assert pos_in == len(s)
        # per-sentry dstloc columns, masked to the sentry's bucket span
        dstl = np.full((nsent, P), 255.0, np.float32)
        for j, (ci, b, _st, _sp) in enumerate(sentries):
            q = int(np.searchsorted(bank_c0, ci, side="right")) - 1
            o, n = int(bucket_off[q, b]), int(seg[q, b])
            lo_p = max(o, ci * P)
            hi_p = min(o + n, (ci + 1) * P)
            dstl[j, lo_p - ci * P : hi_p - ci * P] = dst_pos[lo_p:hi_p]
        out.append((idx_stream, dstl))
    return {"totch": totch, "specs": specs, "sentries": sentries}, out


def _pack_idx16(idx_stream, specs):
    """[128, totch*8] int16: per gather instruction local index j lives at
    row 16 + j%16 (HW) and j%16 (CoreSim), column c0*8 + j//16."""
    totch = len(idx_stream) // P
    arr = np.zeros((P, totch * 8), np.int16)
    for (_q, c0, nch) in specs:
        seg = idx_stream[c0 * P : (c0 + nch) * P]
        w = seg.reshape(-1, 16).T  # [16, nch*8]
        arr[0:16, c0 * 8 : (c0 + nch) * 8] = w
        arr[16:32, c0 * 8 : (c0 + nch) * 8] = w
    return arr


# ----------------------------------------------------------------------------
# device kernel builder
# ----------------------------------------------------------------------------

def _build_nc(n_nodes, npc, nblk, bank_rows, tables, split=True):
    """tables: per graph dict(totch, specs, sentries)"""
    npc_pad = nblk * P
    last_rows = npc - (nblk - 1) * P

    nc = bacc.Bacc(None, target_bir_lowering=False, debug=False)

    # ---- parameters (per core) ----
    XB = 7  # x-tile DMA batch (blocks per load); nblk padded to a multiple
    ngrp = -(-nblk // XB)
    xtb = [nc.declare_dram_parameter(f"xtb{g}", [ngrp, P, 2 * XB * P], F32, isOutput=False)
           for g in range(2)]
    deg_in = [nc.declare_dram_parameter(f"deg{g}", [P, nblk], F32, isOutput=False)
              for g in range(2)]
    dstl_in = [nc.declare_dram_parameter(f"dstloc{g}", [P, len(tables[g]["sentries"])], F32, isOutput=False)
               for g in range(2)]
    idx_in = [nc.declare_dram_parameter(f"idx{g}", [P, tables[g]["totch"] * 8], I16, isOutput=False)
              for g in range(2)]
    w1p_in = nc.declare_dram_parameter("w1p", [P, 2 * HID], F32, isOutput=False)
    w2_in = nc.declare_dram_parameter("w2", [HID, HID], F32, isOutput=False)
    b1_in = nc.declare_dram_parameter("b1t", [P, HID], F32, isOutput=False)
    b2_in = nc.declare_dram_parameter("b2t", [P, HID], F32, isOutput=False)
    iota_in = nc.declare_dram_parameter("iota", [P, P], F32, isOutput=False)
    ident_in = nc.declare_dram_parameter("ident", [P, P], F32, isOutput=False)
    ones_in = nc.declare_dram_parameter("ones", [P, P], F32, isOutput=False)
    zout = nc.declare_dram_parameter("zout", [2, P, nblk * HID], F32, isOutput=True)

    # ---- internal DRAM ----
    g_shard = [[nc.dram_tensor(f"gshard{g}_{l}", [npc, HID], F32) for l in range(2)]
               for g in range(2)]
    g_full = [[nc.dram_tensor(f"gfull{g}_{l}", [n_nodes, HID], F32, addr_space="Shared")
               for l in range(2)] for g in range(2)]
    g_mir = [[nc.dram_tensor(f"gmir{g}_{l}", [n_nodes, HID], F32) for l in range(2)]
             for g in range(2)]
    stats_in = nc.dram_tensor("stats_in", [1, 4 * HID], F32)
    stats_out = nc.dram_tensor("stats_out", [1, 4 * HID], F32, addr_space="Shared")

    rg = [list(range(CORES))]

    with tile.TileContext(nc) as tc:
        with (
            tc.tile_pool(name="const", bufs=1) as cpool,
            tc.tile_pool(name="acc", bufs=1) as apool,
            tc.tile_pool(name="work", bufs=3) as wpool,
            tc.tile_pool(name="blk", bufs=4) as bpool,
            tc.tile_pool(name="psA", bufs=2, space="PSUM") as psA,
            tc.tile_pool(name="psTr", bufs=1, space="PSUM") as psTr,
            tc.tile_pool(name="psAgg", bufs=3, space="PSUM") as psAgg,
            tc.tile_pool(name="psSm", bufs=1, space="PSUM") as psSm,
        ):
            # ---- constants ----
            w1p = cpool.tile([P, 2 * HID], F32)
            nc.sync.dma_start(w1p[:], w1p_in[:])
            w2sb = cpool.tile([HID, HID], F32)
            nc.sync.dma_start(w2sb[:], w2_in[:])
            b1sb = cpool.tile([P, HID], F32)
            nc.sync.dma_start(b1sb[:], b1_in[:])
            b2sb = cpool.tile([P, HID], F32)
            nc.sync.dma_start(b2sb[:], b2_in[:])
            iota = cpool.tile([P, P], F32)
            nc.sync.dma_start(iota[:], iota_in[:])
            ident = cpool.tile([P, P], F32)
            nc.sync.dma_start(ident[:], ident_in[:])
            ones = cpool.tile([P, P], F32)
            nc.sync.dma_start(ones[:], ones_in[:])
            ones_col = ones[:, 0:1]         # [128, 1] of ones
            ones_row = ones[0:1, :]         # [1, 128] of ones

            dinv = []
            for g in range(2):
                dt = cpool.tile([P, nblk], F32, tag=f"deg{g}")
                nc.sync.dma_start(dt[:], deg_in[g][:])
                sq = cpool.tile([P, nblk], F32, tag=f"dsq{g}")
                nc.scalar.activation(sq[:], dt[:], mybir.ActivationFunctionType.Sqrt)
                dv = cpool.tile([P, nblk], F32, tag=f"dinv{g}")
                nc.vector.reciprocal(dv[:], sq[:])
                dinv.append(dv)

            accB = [apool.tile([P, nblk * HID], F32, tag=f"accB{g}", name=f"accB{g}") for g in range(2)]
            accC = [apool.tile([P, nblk * HID], F32, tag=f"accC{g}", name=f"accC{g}") for g in range(2)]

            def rows_of(b):
                return last_rows if b == nblk - 1 else P

            # ---- phase A: g0 = (x @ W1) * dinv, allgather ----
            # graph 1's AllGather trigger is NOT emitted here: on the in-order
            # GpSimd engine it would precede graph 0's gathers and stall them
            # until all of phase A(g1) completes. It is injected into the
            # gather stream below instead.
            ag_pending = {}
            for g in range(2):
                for b in range(nblk):
                    if b % XB == 0:
                        xt4 = wpool.tile([P, 2 * XB * P], F32, tag="xt", bufs=3)
                        nc.scalar.dma_start(xt4[:], xtb[g][b // XB])
                    ph = psA.tile([P, HID], F32, tag="hps")
                    for k in range(2):
                        ko = ((b % XB) * 2 + k) * P
                        xt = xt4[:, ko : ko + P]
                        nc.tensor.matmul(
                            out=ph[:], lhsT=xt, rhs=w1p[:, k * HID : (k + 1) * HID],
                            start=(k == 0), stop=(k == 1))
                    gblk = accB[g][:, b * HID : (b + 1) * HID]
                    nc.scalar.activation(gblk, ph[:],
                                         mybir.ActivationFunctionType.Copy,
                                         scale=dinv[g][:, b : b + 1])
                    r = rows_of(b)
                    nc.sync.dma_start(g_shard[g][0][b * P : b * P + r, :], accB[g][:r, b * HID : (b + 1) * HID])
                def _emit_ag(g=g):
                    nc.gpsimd.collective_compute(
                        "AllGather", mybir.AluOpType.bypass, replica_groups=rg,
                        ins=[g_shard[g][0][:]], outs=[g_full[g][0][:]])
                if g == 0:
                    _emit_ag()
                else:
                    ag_pending[1] = _emit_ag

            # ---- aggregation emitter ----
            dstl_tiles = {}
            for g in range(2):
                dt_ = cpool.tile([P, len(tables[g]["sentries"])], F32, tag=f"dstl{g}")
                nc.sync.dma_start(dt_[:], dstl_in[g][:])
                dstl_tiles[g] = dt_

            def aggregate(g, layer, acc, inject=None):
                """acc[:, b*64:(b+1)*64] += segment_sum of gathered g rows.
                inject: {spec_index: callback} emitted right after that gather
                (used to slot collective triggers into the Pool engine stream
                without head-of-line blocking)."""
                if int(os.environ.get("KERNEL_NO_AGG", "0")):
                    return
                t = tables[g]
                specs, sentries = t["specs"], t["sentries"]
                dstl = dstl_tiles[g]
                table = g_full[g][layer]
                gt = {}
                spec_i = 0
                stile = None
                sbase = 0
                ps = None
                nsent = len(sentries)
                for j, (ci, b, st, sp) in enumerate(sentries):
                    # emit every gather whose chunk range we have now reached
                    while spec_i < len(specs) and specs[spec_i][1] <= ci:
                        q, c0, nch = specs[spec_i]
                        it = wpool.tile([P, GCHUNK * 8], I16, tag="idx", bufs=6)
                        nc.sync.dma_start(it[:, : nch * 8], idx_in[g][:, c0 * 8 : (c0 + nch) * 8])
                        gtile = wpool.tile([P, GCHUNK * HID], F32, tag="gt", bufs=6)
                        nc.gpsimd.dma_gather(
                            gtile[:, : nch * HID].rearrange("p (c d) -> p c d", c=nch),
                            table[q * bank_rows : (q + 1) * bank_rows, :],
                            it[:, : nch * 8], nch * P, nch * P, HID)
                        gt = {"tile": gtile, "c0": c0}
                        if inject and spec_i in inject:
                            inject.pop(spec_i)()
                        spec_i += 1
                    if j % SGROUP == 0:
                        ns = min(SGROUP, nsent - j)
                        stile = wpool.tile([P, SGROUP * P], F32, tag="stile")
                        s3 = stile[:, : ns * P].rearrange("p (c j) -> p c j", c=ns)
                        nc.vector.tensor_tensor(
                            out=s3,
                            in0=dstl[:, j : j + ns][:, :, None].to_broadcast([P, ns, P]),
                            in1=iota[:, None, :].to_broadcast([P, ns, P]),
                            op=mybir.AluOpType.is_equal)
                        sbase = j
                    if st:
                        ps = psAgg.tile([P, HID], F32, tag="aggps")
                    co = ci - gt["c0"]
                    nc.tensor.matmul(
                        out=ps[:],
                        lhsT=stile[:, (j - sbase) * P : (j - sbase + 1) * P],
                        rhs=gt["tile"][:, co * HID : (co + 1) * HID],
                        start=st, stop=sp, skip_group_check=True)
                    if sp:
                        sl = acc[:, b * HID : (b + 1) * HID]
                        nc.vector.tensor_tensor(out=sl, in0=sl, in1=ps[:],
                                                op=mybir.AluOpType.add)
                if inject:
                    for cb in list(inject.values()):
                        cb()
                    inject.clear()

            # ---- phase B: layer-1 aggregation, relu, @W2, allgather ----
            for g in range(2):
                inj = None
                if g == 0:
                    inj = {min(20, len(tables[0]["specs"]) - 1): ag_pending.pop(1)}
                aggregate(g, 0, accB[g], inject=inj)
                for b in range(nblk):
                    sl = accB[g][:, b * HID : (b + 1) * HID]
                    t1 = bpool.tile([P, HID], F32, tag="t1")
                    nc.scalar.activation(t1[:], sl, mybir.ActivationFunctionType.Copy,
                                         scale=dinv[g][:, b : b + 1])
                    t2 = bpool.tile([P, HID], F32, tag="t2")
                    nc.vector.tensor_tensor(out=t2[:], in0=t1[:], in1=b1sb[:],
                                            op=mybir.AluOpType.add)
                    r = bpool.tile([P, HID], F32, tag="t3")
                    nc.scalar.activation(r[:], t2[:], mybir.ActivationFunctionType.Relu)
                    trp = psTr.tile([HID, P], F32, tag="trps")
                    nc.tensor.transpose(out=trp[:], in_=r[:], identity=ident[:])
                    trs = bpool.tile([HID, P], F32, tag="trs")
                    nc.vector.tensor_copy(trs[:], trp[:])
                    p2 = psA.tile([P, HID], F32, tag="hps")
                    nc.tensor.matmul(out=p2[:], lhsT=trs[:], rhs=w2sb[:],
                                     start=True, stop=True)
                    g2b = accC[g][:, b * HID : (b + 1) * HID]
                    nc.scalar.activation(g2b, p2[:], mybir.ActivationFunctionType.Copy,
                                         scale=dinv[g][:, b : b + 1])
                    rr = rows_of(b)
                    nc.sync.dma_start(g_shard[g][1][b * P : b * P + rr, :], accC[g][:rr, b * HID : (b + 1) * HID])
                nc.gpsimd.collective_compute(
                    "AllGather", mybir.AluOpType.bypass, replica_groups=rg,
                    ins=[g_shard[g][1][:]], outs=[g_full[g][1][:]])

            # ---- phase C: layer-2 aggregation, out2, stats ----
            stats_sb = cpool.tile([1, 4 * HID], F32, tag="stats_sb")
            for g in range(2):
                aggregate(g, 1, accC[g])
                pst = psSm.tile([1, 2 * HID], F32, tag="pstats", name="pst")
                for b in range(nblk):
                    sl = accC[g][:, b * HID : (b + 1) * HID]
                    t1 = bpool.tile([P, HID], F32, tag="t1")
                    nc.scalar.activation(t1[:], sl, mybir.ActivationFunctionType.Copy,
                                         scale=dinv[g][:, b : b + 1])
                    o2sq = bpool.tile([P, 2 * HID], F32, tag="t2")
                    o2 = o2sq[:, :HID]
                    sq = o2sq[:, HID:]
                    nc.vector.tensor_tensor(out=o2, in0=t1[:], in1=b2sb[:],
                                            op=mybir.AluOpType.add)
                    nc.vector.tensor_tensor(out=sq, in0=o2, in1=o2,
                                            op=mybir.AluOpType.mult)
                    # overwrite accC block in place with the final conv2 output
                    nc.scalar.activation(sl, o2, mybir.ActivationFunctionType.Copy)
                    rr = rows_of(b)
                    nc.tensor.matmul(out=pst[:], lhsT=ones_col[:rr], rhs=o2sq[:rr, :],
                                     start=(b == 0), stop=(b == nblk - 1),
                                     skip_group_check=True)
                nc.vector.tensor_copy(stats_sb[:, 2 * HID * g : 2 * HID * (g + 1)], pst[:])
            nc.sync.dma_start(stats_in[:], stats_sb[:])
            nc.gpsimd.collective_compute(
                "AllReduce", mybir.AluOpType.add, replica_groups=rg,
                ins=[stats_in[:]], outs=[stats_out[:]])
            stats_rx = cpool.tile([1, 4 * HID], F32, tag="stats_rx")
            nc.sync.dma_start(stats_rx[:], stats_out[:])

            # ---- z-score ----
            n_f = float(n_nodes)
            for g in range(2):
                srow = stats_rx[:, 2 * HID * g : 2 * HID * g + HID]
                qrow = stats_rx[:, 2 * HID * g + HID : 2 * HID * (g + 1)]
                mean = cpool.tile([1, HID], F32, tag=f"mean{g}")
                nc.scalar.activation(mean[:], srow, mybir.ActivationFunctionType.Copy,
                                     scale=1.0 / n_f)
                s2 = cpool.tile([1, HID], F32, tag=f"s2_{g}")
                nc.vector.tensor_tensor(out=s2[:], in0=srow, in1=srow,
                                        op=mybir.AluOpType.mult)
                s2n = cpool.tile([1, HID], F32, tag=f"s2n{g}")
                nc.scalar.activation(s2n[:], s2[:], mybir.ActivationFunctionType.Copy,
                                     scale=1.0 / n_f)
                v = cpool.tile([1, HID], F32, tag=f"v{g}")
                nc.vector.tensor_tensor(out=v[:], in0=qrow, in1=s2n[:],
                                        op=mybir.AluOpType.subtract)
                stdv = cpool.tile([1, HID], F32, tag=f"std{g}")
                nc.scalar.activation(stdv[:], v[:], mybir.ActivationFunctionType.Sqrt,
                                     scale=1.0 / (n_f - 1.0))
                rstd = cpool.tile([1, HID], F32, tag=f"rstd{g}")
                nc.vector.reciprocal(rstd[:], stdv[:])
                pb = psSm.tile([P, 2 * HID], F32, tag="bcast")
                pm = pb[:, :HID]
                pr = pb[:, HID:]
                nc.tensor.matmul(out=pm, lhsT=ones_row, rhs=mean[:],
                                 start=True, stop=True, skip_group_check=True)
                nc.tensor.matmul(out=pr, lhsT=ones_row, rhs=rstd[:],
                                 start=True, stop=True, skip_group_check=True)
                ZB = 14  # blocks per z-score slab (98 = 7*14)
                for s in range(0, nblk, ZB):
                    nb2 = min(ZB, nblk - s)
                    slab = accC[g][:, s * HID : (s + nb2) * HID]
                    s3 = slab.rearrange("p (c f) -> p c f", c=nb2)
                    z1 = bpool.tile([P, ZB * HID], F32, tag="z1", bufs=2)
                    z13 = z1[:, : nb2 * HID].rearrange("p (c f) -> p c f", c=nb2)
                    nc.vector.tensor_tensor(
                        out=z13, in0=s3,
                        in1=pm[:, None, :].to_broadcast([P, nb2, HID]),
                        op=mybir.AluOpType.subtract)
                    z2 = bpool.tile([P, ZB * HID], F32, tag="z2", bufs=2)
                    z23 = z2[:, : nb2 * HID].rearrange("p (c f) -> p c f", c=nb2)
                    nc.vector.tensor_tensor(
                        out=z23, in0=z13,
                        in1=pr[:, None, :].to_broadcast([P, nb2, HID]),
                        op=mybir.AluOpType.mult)
                    nc.sync.dma_start(
                        zout[g, :, s * HID : (s + nb2) * HID],
                        z2[:, : nb2 * HID])

    nc.compile()
    if split:
        _split_waits(nc, max_waits=1)
    return nc


# ----------------------------------------------------------------------------
# wait-splitting post-pass (walrus rejects >1 sync wait per instruction here)
# ----------------------------------------------------------------------------

def _split_waits(nc, max_waits=1):
    inserted = 0
    for blk in nc.main_func.blocks:
        bb = blk if hasattr(blk, "instructions") else blk.bb
        new_list = []
        for ins in bb.instructions:
            si = ins.sync_info
            waits = list(si.on_wait) if (si and si.on_wait) else []
            if len(waits) > max_waits:
                keep = waits[-max_waits:]
                extra = waits[:-max_waits]
                for i in range(0, len(extra), max_waits):
                    chunk = extra[i : i + max_waits]
                    nop = mybir.InstNoOp(
                        name=nc.get_next_instruction_name(),
                        engine=ins.engine, ins=[], outs=[], text_hint="wait_split")
                    nop.sync_info = mybir.SyncInfo(on_wait=chunk, on_update=[])
                    new_list.append(nop)
                    inserted += 1
                si.on_wait = keep
            new_list.append(ins)
        bb.instructions[:] = new_list
    return inserted


# ----------------------------------------------------------------------------
# host wrapper
# ----------------------------------------------------------------------------

def _prepare(x1, edge_index1, x2, edge_index2, W1, b1, W2, b2, n_nodes):
    npc = n_nodes // CORES
    nblk = -(-npc // P)
    npc_pad = nblk * P
    bank_rows = -(-n_nodes // BANKS)
    assert bank_rows <= 32767

    graphs = [(np.asarray(x1), np.asarray(edge_index1)),
              (np.asarray(x2), np.asarray(edge_index2))]
    tables = []
    per_core_arrays = [dict() for _ in range(CORES)]
    for g, (x, ei) in enumerate(graphs):
        src0 = np.asarray(ei[0], dtype=np.int64)
        dst0 = np.asarray(ei[1], dtype=np.int64)
        pos = _balance(src0, dst0, n_nodes, npc, nblk, bank_rows)
        inv = np.empty(n_nodes, np.int64)
        inv[pos] = np.arange(n_nodes)
        src = pos[src0]
        dst = pos[dst0]
        deg = np.bincount(dst, minlength=n_nodes).astype(np.float32) + 1.0
        table, routed = _route_graph(src, dst, n_nodes, npc, nblk, bank_rows)
        table["pos"] = pos
        tables.append(table)
        x = np.asarray(x, dtype=np.float32)
        for c in range(CORES):
            idx_stream, dstl = routed[c]
            d = per_core_arrays[c]
            XB = 7
            ngrp = -(-nblk // XB)
            xp = np.zeros((ngrp * XB * P, IN_DIM), np.float32)
            xp[:npc] = x[inv[c * npc : (c + 1) * npc]]
            # [grp, q, b_in_grp, k, p] so each group is one 2D SBUF-layout DMA
            d[f"xtb{g}"] = np.ascontiguousarray(
                xp.reshape(ngrp, XB, P, 2, P).transpose(0, 4, 1, 3, 2)
                .reshape(ngrp, P, 2 * XB * P))
            degp = np.ones(npc_pad, np.float32)
            degp[:npc] = deg[c * npc : (c + 1) * npc]
            d[f"deg{g}"] = np.ascontiguousarray(degp.reshape(nblk, P).T)
            d[f"dstloc{g}"] = np.ascontiguousarray(dstl.T)
            d[f"idx{g}"] = _pack_idx16(idx_stream, table["specs"])

    W1 = np.asarray(W1, np.float32)
    w1p = np.zeros((P, 2 * HID), np.float32)
    w1p[:, :HID] = W1[:P]
    w1p[:, HID:] = W1[P:]
    shared = {
        "w1p": w1p,
        "w2": np.asarray(W2, np.float32),
        "b1t": np.broadcast_to(np.asarray(b1, np.float32), (P, HID)).copy(),
        "b2t": np.broadcast_to(np.asarray(b2, np.float32), (P, HID)).copy(),
        "iota": np.tile(np.arange(P, dtype=np.float32), (P, 1)),
        "ident": np.eye(P, dtype=np.float32),
        "ones": np.ones((P, P), np.float32),
    }
    for d in per_core_arrays:
        d.update(shared)
    return tables, per_core_arrays, npc, nblk, bank_rows



def _install_profile_shim():
    """ctypes NTFF hook for run_bass_kernel_spmd(trace=True) under axon."""
    import contextlib
    import ctypes
    import types
    if "antenv.axon_hooks" in sys.modules:
        return
    try:
        lib = ctypes.CDLL("/opt/axon/libaxon_pjrt.so")
        lib.axon_start_nrt_profile.argtypes = [ctypes.POINTER(ctypes.c_int64), ctypes.c_size_t]
        lib.axon_start_nrt_profile.restype = ctypes.c_int64
        lib.axon_stop_nrt_profile.argtypes = [ctypes.c_char_p]
        lib.axon_stop_nrt_profile.restype = ctypes.c_int64
    except (OSError, AttributeError):
        return

    @contextlib.contextmanager
    def _hook(output_dir, device_ids):
        import jax
        jax.devices()
        if device_ids:
            ids = (ctypes.c_int64 * len(device_ids))(*device_ids)
            rc = lib.axon_start_nrt_profile(ids, len(device_ids))
        else:
            rc = lib.axon_start_nrt_profile(None, 0)
        if rc != 0:
            raise RuntimeError(f"axon_start_nrt_profile rc={rc}")
        try:
            yield
        finally:
            n = lib.axon_stop_nrt_profile(str(output_dir).encode())
            print(f"ntff profile: {n} file(s) -> {output_dir}", file=sys.stderr)

    mod = types.ModuleType("antenv.axon_hooks")
    mod.get_axon_ntff_profile_hook = lambda: _hook
    mod.set_axon_ntff_profile_hook = lambda h: None
    sys.modules["antenv.axon_hooks"] = mod

    from concourse import bass_utils
    bass_utils.upload_artifacts = lambda tmpdir: f"local:{tmpdir}"

_NC_CACHE = {}


def _run(x1, edge_index1, x2, edge_index2, W1, b1, W2, b2, n_nodes, trace=False):
    global LAST_EXEC_NS
    tables, in_maps, npc, nblk, bank_rows = _prepare(
        x1, edge_index1, x2, edge_index2, W1, b1, W2, b2, n_nodes)

    sim_mode = bool(int(os.environ.get("KERNEL_SIM", "0")))
    key = (n_nodes, sim_mode,
           tuple(tables[0]["sentries"]), tuple(tables[0]["specs"]),
           tuple(tables[1]["sentries"]), tuple(tables[1]["specs"]))
    if key not in _NC_CACHE:
        _NC_CACHE[key] = _build_nc(n_nodes, npc, nblk, bank_rows, tables,
                                   split=not sim_mode)
    nc = _NC_CACHE[key]

    def _unscramble(zraw):
        # [2, P, nblk*HID] partition-major -> [2, npc, HID]
        z = np.asarray(zraw).reshape(2, P, nblk, HID).transpose(0, 2, 1, 3)
        return z.reshape(2, nblk * P, HID)[:, :npc]

    if sim_mode:
        from concourse import bass_interp
        sim = bass_interp.MultiCoreSim(nc, CORES)
        for c in range(CORES):
            for k, v in in_maps[c].items():
                sim.cores[c].tensor(k)[:] = v
        sim.simulate()
        outs = [_unscramble(sim.cores[c].mem_tensor("zout").reshape(2, P, nblk * HID))
                for c in range(CORES)]
        z1 = np.concatenate([o[0] for o in outs], axis=0)[tables[0]["pos"]]
        z2 = np.concatenate([o[1] for o in outs], axis=0)[tables[1]["pos"]]
        return z1, z2

    kwargs = {}
    if trace:
        _install_profile_shim()
        kwargs["trace"] = True
    res = run_bass_kernel_spmd(nc, in_maps, core_ids=list(range(CORES)), **kwargs)
    LAST_EXEC_NS = res.exec_time_ns
    outs = [_unscramble(res.results[c]["zout"]) for c in range(CORES)]
    z1 = np.concatenate([o[0] for o in outs], axis=0)[tables[0]["pos"]]
    z2 = np.concatenate([o[1] for o in outs], axis=0)[tables[1]["pos"]]
    return z1, z2


def kernel(x1, edge_index1, x2, edge_index2, W1, b1, W2, b2):
    trace = bool(int(os.environ.get("KERNEL_TRACE", "0")))
    return _run(x1, edge_index1, x2, edge_index2, W1, b1, W2, b2,
                n_nodes=100000, trace=trace)

